# revision 9
# baseline (speedup 1.0000x reference)
"""Bipartite GCN stack (2 layers) on 8 Trainium2 NeuronCores.

Strategy (graph/data parallel, destination-sharded):
  - Layer-0 forward: every core computes the full WH0 = H_src @ W_fwd0
    (redundantly; cheaper than all-gathering the 64MB table), then
    processes the ~1/8 of edges whose destination (row) falls in its
    4096-target shard.  segment_sum is done by gathering WH0 rows with
    indirect DMA (128 rows / instruction) and reducing each 128-edge
    tile into PSUM with a selection-matrix matmul (S[e,d] = val[e] if
    dst_local[e]==d).  A 257th ones-column of the gathered tile yields
    the degree in the same matmul.
  - BatchNorm: per-core partial sums via bn_stats in feature-major
    layout, 2KB AllReduce, scale/shift applied in feature-major form
    (which is exactly the lhsT layout the next dense matmul needs).
  - Backward (layer 0 only; the layer-1 backward output is dead):
    AllGather of WHb (32MB), edges sharded by source, same reduction.
  - Layer-1 forward: AllGather of WH1 (64MB), reuse of the layer-0
    target degrees, residual in feature-major form, BN, output.

Host-side work is limited to sharding/permutation of the edge index
arrays and layout transforms (transposes) of inputs/outputs; all
floating-point math runs on the NeuronCores.
"""

import math

import numpy as np

P = 128
D_FIXED = 256
EPS = 1e-5
NCORES = 8

# matmul input dtype for all PE ops: "float32" (exact, 4 cyc/row) or
# "float32r" (TF32-like, 1 cyc/row at N>=256)
_MM_DT = "float32"


# ----------------------------------------------------------------- host prep


def _edge_plan(dst, gidx, vals, n_dst, ncores):
    """Partition edges by destination shard, group by 128-row dst tile,
    pad each (core, tile) group to the max tile count over cores.

    Returns (ntiles_per_dsttile, per-core [P, NF] arrays: gather-index
    (int32), val (f32), dst_local (f32))."""
    dst_sh = n_dst // ncores
    nt = dst_sh // P
    core_of = dst // dst_sh
    tile_of = (dst % dst_sh) // P
    dl = (dst % P).astype(np.float32)

    order = np.lexsort((tile_of, core_of))
    so_core = core_of[order]
    so_tile = tile_of[order]
    so_gidx = gidx[order].astype(np.int32)
    so_val = vals[order].astype(np.float32)
    so_dl = dl[order]

    counts = np.bincount(core_of * nt + tile_of, minlength=ncores * nt).reshape(
        ncores, nt
    )
    ntile = np.maximum(1, np.ceil(counts.max(axis=0) / P).astype(np.int64))  # [nt]
    nf = int(ntile.sum())
    tile_off = np.concatenate([[0], np.cumsum(ntile)])  # [nt+1] in tiles

    g_arr = np.zeros((ncores, nf * P), dtype=np.int32)
    v_arr = np.zeros((ncores, nf * P), dtype=np.float32)
    d_arr = np.zeros((ncores, nf * P), dtype=np.float32)

    grp_start = np.concatenate([[0], np.cumsum(counts.reshape(-1))])
    for c in range(ncores):
        for t in range(nt):
            gi = c * nt + t
            s, e = grp_start[gi], grp_start[gi + 1]
            n = e - s
            o = tile_off[t] * P
            g_arr[c, o : o + n] = so_gidx[s:e]
            v_arr[c, o : o + n] = so_val[s:e]
            d_arr[c, o : o + n] = so_dl[s:e]
    # SBUF layout [p, j]: edge j*P+p
    g_dev = [np.ascontiguousarray(g_arr[c].reshape(nf, P).T) for c in range(ncores)]
    v_dev = [np.ascontiguousarray(v_arr[c].reshape(nf, P).T) for c in range(ncores)]
    d_dev = [np.ascontiguousarray(d_arr[c].reshape(nf, P).T) for c in range(ncores)]
    return [int(x) for x in ntile], g_dev, v_dev, d_dev


# ----------------------------------------------------------------- bass build


def _install_drain_patch():
    """walrus in this env allows only 1 sem-wait on a TPB_CTRL Drain; split
    the Tile tail drain's waits across multiple drains."""
    import concourse.mybir as mybir
    import concourse.tile as _tile
    from concourse.vector_clock import ScopedClock

    if getattr(_tile.TileContext, "_drain_split_patched", False):
        return

    def _split_drain_and_barrier(self, tick_clock, wait_clock):
        nc = self.nc
        drain_inst = nc.sync.drain()
        wait_clock.add_sem_waits(
            drain_inst.ins, ScopedClock({None: tick_clock.global_clock})
        )
        si = drain_inst.ins.sync_info
        waits = list(si.on_wait) if si and si.on_wait else []
        if len(waits) > 1:
            si.on_wait = waits[:1]
            drain_inst.ins.sync_info = si
            for i in range(1, len(waits)):
                extra = nc.sync.drain()
                esi = extra.ins.sync_info
                upd = list(esi.on_update) if esi and esi.on_update else []
                extra.ins.sync_info = mybir.SyncInfo(
                    on_wait=[waits[i]], on_update=upd
                )
        nc.all_engine_barrier()
        assert self.sems is not None
        popped = nc._tile_sem_poison_stack.pop()
        assert popped is self._sem_poison
        nc.clear_and_free_semaphores(list(self.sems.allocated().values()))
        nc.all_engine_barrier()

    _tile.TileContext._drain_and_barrier = _split_drain_and_barrier

    # Split >1 sem-waits on ANY instruction: walrus setupSyncWait in this
    # env accepts a single wait command per instruction.  Extra waits are
    # moved onto same-engine InstNoOps emitted immediately before.
    _orig_add = _tile.TileContext._add_instruction

    def _add_instruction_split(self, inst):
        si = inst.sync_info
        waits = list(si.on_wait) if si and si.on_wait else []
        if len(waits) > 1 and inst.engine != mybir.EngineType.Unassigned:
            for w in waits[:-1]:
                nop = mybir.InstNoOp(
                    name=self.nc.get_next_instruction_name(), ins=[], outs=[]
                )
                nop.engine = inst.engine
                nop.sync_info = mybir.SyncInfo(on_wait=[w], on_update=[])
                _orig_add(self, nop)
            si.on_wait = waits[-1:]
            inst.sync_info = si
        _orig_add(self, inst)

    _tile.TileContext._add_instruction = _add_instruction_split
    _tile.TileContext._drain_split_patched = True


def _build_program(n_tgt, n_src, n1, nb, taps=False):
    """Build the SPMD bass program (identical on all 8 cores)."""
    from contextlib import ExitStack

    import concourse.bass as bass
    import concourse.mybir as mybir
    import concourse.tile as tile
    from concourse.masks import make_identity

    _install_drain_patch()

    dt = mybir.dt
    f32 = dt.float32
    i32 = dt.int32
    mm_dt = getattr(dt, _MM_DT)
    D = D_FIXED
    DC = D // P  # feature chunks
    tgt_sh = n_tgt // NCORES
    src_sh = n_src // NCORES
    NT = tgt_sh // P
    NS = src_sh // P
    NF = sum(n1)
    NB = sum(nb)
    GR = 8  # gather ring slots
    AluOp = mybir.AluOpType
    Act = mybir.ActivationFunctionType
    rg = [list(range(NCORES))]

    def mm(ap):
        return ap.bitcast(mm_dt) if mm_dt != f32 else ap

    nc = bass.Bass("TRN2", target_bir_lowering=False, debug=False, num_devices=NCORES)

    dram_t = nc.dram_tensor
    HsrcT = dram_t("HsrcT", [D, n_src], f32, kind="ExternalInput").ap()
    W0 = dram_t("W0", [D, D], f32, kind="ExternalInput").ap()
    Wb = dram_t("Wb", [D, D], f32, kind="ExternalInput").ap()
    W1 = dram_t("W1", [D, D], f32, kind="ExternalInput").ap()
    b0_h = dram_t("b0", [1, D], f32, kind="ExternalInput")
    bb_h = dram_t("bb", [1, D], f32, kind="ExternalInput")
    b1_h = dram_t("b1", [1, D], f32, kind="ExternalInput")
    g1T = dram_t("g1T", [P, DC], f32, kind="ExternalInput").ap()
    be1T = dram_t("be1T", [P, DC], f32, kind="ExternalInput").ap()
    g2T = dram_t("g2T", [P, DC], f32, kind="ExternalInput").ap()
    be2T = dram_t("be2T", [P, DC], f32, kind="ExternalInput").ap()
    iota_d = dram_t("iota", [P, P], f32, kind="ExternalInput").ap()
    emb = dram_t("emb", [tgt_sh, D], f32, kind="ExternalInput").ap()
    fe_col = dram_t("fe_col", [P, NF], i32, kind="ExternalInput").ap()
    fe_val = dram_t("fe_val", [P, NF], f32, kind="ExternalInput").ap()
    fe_dl = dram_t("fe_dl", [P, NF], f32, kind="ExternalInput").ap()
    be_row = dram_t("be_row", [P, NB], i32, kind="ExternalInput").ap()
    be_val = dram_t("be_val", [P, NB], f32, kind="ExternalInput").ap()
    be_dl = dram_t("be_dl", [P, NB], f32, kind="ExternalInput").ap()
    outT = dram_t("outT", [D, tgt_sh], f32, kind="ExternalOutput").ap()
    if taps:
        dbg_wh0 = dram_t("dbg_wh0", [n_src, D], f32, kind="ExternalOutput").ap()
        dbg_x1T = dram_t("dbg_x1T", [D, tgt_sh], f32, kind="ExternalOutput").ap()
        dbg_whb = dram_t("dbg_whb", [n_tgt, D], f32, kind="ExternalOutput").ap()
        dbg_wh1 = dram_t("dbg_wh1", [n_src, D], f32, kind="ExternalOutput").ap()
        dbg_st1 = dram_t("dbg_st1", [P, 2 * DC], f32, kind="ExternalOutput").ap()

    with tile.TileContext(nc) as tc, ExitStack() as ctx:
        dram = ctx.enter_context(tc.tile_pool(name="dram", bufs=1, space="DRAM"))
        WH0_full = dram.tile([n_src, D], f32)
        WHb_loc = dram.tile([tgt_sh, D], f32)
        WHb_full = dram.tile([n_tgt, D], f32, addr_space="Shared")
        WH1_loc = dram.tile([src_sh, D], f32)
        WH1_full = dram.tile([n_src, D], f32, addr_space="Shared")
        st1_in = dram.tile([P, 2 * DC], f32)
        st1_out = dram.tile([P, 2 * DC], f32, addr_space="Shared")
        st2_in = dram.tile([P, 2 * DC], f32)
        st2_out = dram.tile([P, 2 * DC], f32, addr_space="Shared")

        consts = ctx.enter_context(tc.tile_pool(name="consts", bufs=1))
        w0t = consts.tile([P, DC, D], f32)
        wbt = consts.tile([P, DC, D], f32)
        w1t = consts.tile([P, DC, D], f32)
        for c in range(DC):
            nc.sync.dma_start(out=w0t[:, c, :], in_=W0[c * P : (c + 1) * P, :])
            nc.sync.dma_start(out=wbt[:, c, :], in_=Wb[c * P : (c + 1) * P, :])
            nc.sync.dma_start(out=w1t[:, c, :], in_=W1[c * P : (c + 1) * P, :])
        b0bc = consts.tile([P, D], f32)
        bbbc = consts.tile([P, D], f32)
        b1bc = consts.tile([P, D], f32)
        for h, t_ in ((b0_h, b0bc), (bb_h, bbbc), (b1_h, b1bc)):
            nc.gpsimd.dma_start(
                out=t_[:], in_=bass.AP(tensor=h, offset=0, ap=[[0, P], [1, D]])
            )
        g1f = consts.tile([P, DC], f32)
        be1f = consts.tile([P, DC], f32)
        g2f = consts.tile([P, DC], f32)
        be2f = consts.tile([P, DC], f32)
        nc.sync.dma_start(out=g1f[:], in_=g1T[:])
        nc.sync.dma_start(out=be1f[:], in_=be1T[:])
        nc.sync.dma_start(out=g2f[:], in_=g2T[:])
        nc.sync.dma_start(out=be2f[:], in_=be2T[:])
        iota_t = consts.tile([P, P], f32)
        nc.sync.dma_start(out=iota_t[:], in_=iota_d[:])
        epst = consts.tile([P, 1], f32)
        nc.vector.memset(epst[:], EPS)
        ident = consts.tile([P, P], f32)
        make_identity(nc, ident[:])

        fcol = consts.tile([P, NF], i32)
        fval = consts.tile([P, NF], f32)
        fdl = consts.tile([P, NF], f32)
        nc.sync.dma_start(out=fcol[:], in_=fe_col[:])
        nc.sync.dma_start(out=fval[:], in_=fe_val[:])
        nc.sync.dma_start(out=fdl[:], in_=fe_dl[:])
        bcol = consts.tile([P, NB], i32)
        bval = consts.tile([P, NB], f32)
        bdl = consts.tile([P, NB], f32)
        nc.sync.dma_start(out=bcol[:], in_=be_row[:])
        nc.sync.dma_start(out=bval[:], in_=be_val[:])
        nc.sync.dma_start(out=bdl[:], in_=be_dl[:])

        # resident state
        xT = consts.tile([P, DC, NT, P], f32)  # H_tgt1 (feature-major)
        x2T = consts.tile([P, DC, NT, P], f32)  # layer-1 pre-BN x
        degc = consts.tile([P, NT], f32)  # clamped target degrees
        gbuf = consts.tile([P, GR, D + 1], f32)  # gather ring (+ones col)
        for s in range(GR):
            nc.vector.memset(gbuf[:, s, D : D + 1], 1.0)

        sv_pool = ctx.enter_context(tc.tile_pool(name="sv", bufs=8))
        ps_acc = ctx.enter_context(tc.tile_pool(name="psacc", bufs=3, space="PSUM"))
        ps_tr = ctx.enter_context(tc.tile_pool(name="pstr", bufs=3, space="PSUM"))
        hrm = ctx.enter_context(tc.tile_pool(name="hrm", bufs=4))
        whs_pool = ctx.enter_context(tc.tile_pool(name="whs", bufs=4))
        lhs_pool = ctx.enter_context(tc.tile_pool(name="lhs", bufs=3))
        misc = ctx.enter_context(tc.tile_pool(name="misc", bufs=6))

        # ---------------- phase A: full WH0 = H_src @ W0 (every core) ----
        SUP = 1024 if n_src % 1024 == 0 else P
        for st in range(n_src // SUP):
            ha = lhs_pool.tile([P, SUP], f32, tag="ha")
            hb = lhs_pool.tile([P, SUP], f32, tag="hb")
            nc.sync.dma_start(out=ha[:], in_=HsrcT[0:P, st * SUP : (st + 1) * SUP])
            nc.sync.dma_start(out=hb[:], in_=HsrcT[P : 2 * P, st * SUP : (st + 1) * SUP])
            for r in range(SUP // P):
                ps = ps_acc.tile([P, D + 1], f32, tag="acc")
                nc.tensor.matmul(
                    out=ps[:, 0:D],
                    lhsT=mm(ha[:, r * P : (r + 1) * P]),
                    rhs=mm(w0t[:, 0, :]),
                    start=True,
                    stop=False,
                )
                nc.tensor.matmul(
                    out=ps[:, 0:D],
                    lhsT=mm(hb[:, r * P : (r + 1) * P]),
                    rhs=mm(w0t[:, 1, :]),
                    start=False,
                    stop=True,
                )
                whs = whs_pool.tile([P, D], f32, tag="whs")
                nc.vector.tensor_copy(out=whs[:], in_=ps[:, 0:D])
                row0 = (st * (SUP // P) + r) * P
                nc.sync.dma_start(out=WH0_full[row0 : row0 + P, :], in_=whs[:])

        # ---------------- gather pass helper ----------------------------
        def gather_pass(
            n_of_t,
            idx_tile,
            val_tile,
            dl_tile,
            table,
            with_deg,
            post,
            jbase0=0,
        ):
            j = jbase0
            for t in range(len(n_of_t)):
                ntile = n_of_t[t]
                ps = ps_acc.tile([P, D + 1], f32, tag="acc", name=f"psg{t}")
                for k in range(ntile):
                    s = j % GR
                    width = D + 1 if with_deg else D
                    nc.gpsimd.indirect_dma_start(
                        out=gbuf[:, s, 0:D],
                        out_offset=None,
                        in_=table[:],
                        in_offset=bass.IndirectOffsetOnAxis(
                            ap=idx_tile[:, j : j + 1], axis=0
                        ),
                    )
                    sv = sv_pool.tile([P, P], f32, name="sv")
                    nc.vector.tensor_tensor(
                        out=sv[:],
                        in0=dl_tile[:, j : j + 1].to_broadcast([P, P]),
                        in1=iota_t[:],
                        op=AluOp.is_equal,
                    )
                    nc.vector.tensor_scalar_mul(sv[:], sv[:], val_tile[:, j : j + 1])
                    nc.tensor.matmul(
                        out=ps[:, 0:width],
                        lhsT=mm(sv[:]),
                        rhs=mm(gbuf[:, s, 0:width]),
                        start=(k == 0),
                        stop=(k == ntile - 1),
                    )
                    j += 1
                post(t, ps)
            return j

        # ---------------- pass B: layer-0 forward ------------------------
        def post_fwd0(t, ps):
            dtmp = misc.tile([P, 1], f32, tag="dtgt")
            nc.vector.tensor_scalar_max(dtmp[:], ps[:, D : D + 1], 1.0)
            nc.vector.reciprocal(degc[:, t : t + 1], dtmp[:])
            h1 = hrm.tile([P, D], f32, tag="h1")
            nc.vector.scalar_tensor_tensor(
                out=h1[:],
                in0=ps[:, 0:D],
                scalar=degc[:, t : t + 1],
                in1=b0bc[:],
                op0=AluOp.mult,
                op1=AluOp.add,
            )
            h2 = hrm.tile([P, D], f32, tag="h2")
            nc.scalar.activation(out=h2[:], in_=h1[:], func=Act.Relu)
            et = misc.tile([P, D], f32, tag="emb")
            nc.sync.dma_start(out=et[:], in_=emb[t * P : (t + 1) * P, :])
            xr = hrm.tile([P, D], f32, tag="xr")
            nc.vector.tensor_add(xr[:], h2[:], et[:])
            for c in range(DC):
                pt = ps_tr.tile([P, P], f32, tag="tr")
                nc.tensor.transpose(
                    out=pt[:], in_=xr[:, c * P : (c + 1) * P], identity=ident[:]
                )
                nc.vector.tensor_copy(out=xT[:, c, t, :], in_=pt[:])

        gather_pass(n1, fcol, fval, fdl, WH0_full, True, post_fwd0)

        # ---------------- BN-1 stats + apply + WHb -----------------------
        def bn_stats_phase(xbuf, count, st_in_sb_name):
            """Per-core sums of x and x^2 per feature -> [P, 2*DC] SBUF."""
            st_sb = misc.tile([P, 2 * DC], f32, name=st_in_sb_name, tag="stats")
            grp = min(512, count)
            ngrp = count // grp
            for c in range(DC):
                bnst = misc.tile([P, ngrp, 6], f32, tag="bnst")
                flat = xbuf[:, c, :, :].rearrange("p a b -> p (a b)")
                for g in range(ngrp):
                    nc.vector.bn_stats(
                        out=bnst[:, g, :], in_=flat[:, g * grp : (g + 1) * grp]
                    )
                mv = misc.tile([P, 2], f32, tag="mv")
                nc.vector.bn_aggr(out=mv[:], in_=bnst[:].rearrange("p a b -> p (a b)"))
                # S = mean*count ; Q = (var + mean^2)*count
                nc.vector.tensor_scalar_mul(
                    st_sb[:, 2 * c : 2 * c + 1], mv[:, 0:1], float(count)
                )
                musq = misc.tile([P, 1], f32, tag="musq")
                nc.vector.tensor_mul(musq[:], mv[:, 0:1], mv[:, 0:1])
                nc.vector.tensor_add(musq[:], musq[:], mv[:, 1:2])
                nc.vector.tensor_scalar_mul(
                    st_sb[:, 2 * c + 1 : 2 * c + 2], musq[:], float(count)
                )
            return st_sb

        def bn_coeffs(st_full_sb, gamma_f, beta_f, total, a_name, b_name):
            """A = gamma*rsqrt(var+eps), B = beta - mu*A  (feature-major)."""
            A = misc.tile([P, DC], f32, name=a_name, tag="bnA")
            B = misc.tile([P, DC], f32, name=b_name, tag="bnB")
            for c in range(DC):
                mu = misc.tile([P, 1], f32, tag="mu")
                nc.vector.tensor_scalar_mul(
                    mu[:], st_full_sb[:, 2 * c : 2 * c + 1], 1.0 / total
                )
                q = misc.tile([P, 1], f32, tag="q")
                nc.vector.tensor_scalar_mul(
                    q[:], st_full_sb[:, 2 * c + 1 : 2 * c + 2], 1.0 / total
                )
                musq = misc.tile([P, 1], f32, tag="musq2")
                nc.vector.tensor_mul(musq[:], mu[:], mu[:])
                var = misc.tile([P, 1], f32, tag="var")
                nc.vector.tensor_tensor(
                    out=var[:], in0=q[:], in1=musq[:], op=AluOp.subtract
                )
                sd = misc.tile([P, 1], f32, tag="sd")
                nc.scalar.activation(
                    out=sd[:], in_=var[:], func=Act.Sqrt, bias=epst[:]
                )
                rstd = misc.tile([P, 1], f32, tag="rstd")
                nc.vector.reciprocal(rstd[:], sd[:])
                nc.vector.tensor_mul(
                    A[:, c : c + 1], gamma_f[:, c : c + 1], rstd[:]
                )
                mA = misc.tile([P, 1], f32, tag="mA")
                nc.vector.tensor_mul(mA[:], mu[:], A[:, c : c + 1])
                nc.vector.tensor_tensor(
                    out=B[:, c : c + 1],
                    in0=beta_f[:, c : c + 1],
                    in1=mA[:],
                    op=AluOp.subtract,
                )
            return A, B

        if taps:
            nc.sync.dma_start(out=dbg_wh0[:], in_=WH0_full[:])
        st1_sb = bn_stats_phase(xT, tgt_sh, "st1_sb")
        nc.sync.dma_start(out=st1_in[:], in_=st1_sb[:])
        nc.gpsimd.collective_compute(
            "AllReduce",
            AluOp.add,
            replica_groups=rg,
            ins=[st1_in[:].opt()],
            outs=[st1_out[:].opt()],
        )
        st1g = misc.tile([P, 2 * DC], f32, tag="stg")
        nc.sync.dma_start(out=st1g[:], in_=st1_out[:])
        A1, B1 = bn_coeffs(st1g, g1f, be1f, n_tgt, "A1", "B1")

        for t in range(NT):
            for c in range(DC):
                nc.vector.scalar_tensor_tensor(
                    out=xT[:, c, t, :],
                    in0=xT[:, c, t, :],
                    scalar=A1[:, c : c + 1],
                    in1=B1[:, c : c + 1].to_broadcast([P, P]),
                    op0=AluOp.mult,
                    op1=AluOp.add,
                )
            ps = ps_acc.tile([P, D + 1], f32, tag="acc", name=f"pswb{t}")
            nc.tensor.matmul(
                out=ps[:, 0:D],
                lhsT=mm(xT[:, 0, t, :]),
                rhs=mm(wbt[:, 0, :]),
                start=True,
                stop=False,
            )
            nc.tensor.matmul(
                out=ps[:, 0:D],
                lhsT=mm(xT[:, 1, t, :]),
                rhs=mm(wbt[:, 1, :]),
                start=False,
                stop=True,
            )
            whs = whs_pool.tile([P, D], f32, tag="whs")
            nc.vector.tensor_copy(out=whs[:], in_=ps[:, 0:D])
            nc.sync.dma_start(out=WHb_loc[t * P : (t + 1) * P, :], in_=whs[:])

        nc.gpsimd.collective_compute(
            "AllGather",
            AluOp.bypass,
            replica_groups=rg,
            ins=[WHb_loc[:].opt()],
            outs=[WHb_full[:].opt()],
        )
        if taps:
            nc.sync.dma_start(out=dbg_st1[:], in_=st1_out[:])
            for c in range(DC):
                nc.sync.dma_start(
                    out=dbg_x1T[c * P : (c + 1) * P, :],
                    in_=xT[:, c, :, :].rearrange("p a b -> p (a b)"),
                )
            nc.sync.dma_start(out=dbg_whb[:], in_=WHb_full[:])

        # ---------------- pass E: layer-0 backward (+ fused WH1) ---------
        def post_bwd(t, ps):
            dtmp = misc.tile([P, 1], f32, tag="dsrc")
            nc.vector.tensor_scalar_max(dtmp[:], ps[:, D : D + 1], 1.0)
            rtmp = misc.tile([P, 1], f32, tag="rsrc")
            nc.vector.reciprocal(rtmp[:], dtmp[:])
            h1 = hrm.tile([P, D], f32, tag="h1")
            nc.vector.scalar_tensor_tensor(
                out=h1[:],
                in0=ps[:, 0:D],
                scalar=rtmp[:],
                in1=bbbc[:],
                op0=AluOp.mult,
                op1=AluOp.add,
            )
            h2 = hrm.tile([P, D], f32, tag="h2")
            nc.scalar.activation(out=h2[:], in_=h1[:], func=Act.Relu)
            hsb = misc.tile([P, DC, P], f32, tag="hsb")
            for c in range(DC):
                pt = ps_tr.tile([P, P], f32, tag="tr")
                nc.tensor.transpose(
                    out=pt[:], in_=h2[:, c * P : (c + 1) * P], identity=ident[:]
                )
                nc.vector.tensor_copy(out=hsb[:, c, :], in_=pt[:])
            ps2 = ps_acc.tile([P, D + 1], f32, tag="acc", name=f"psw1{t}")
            nc.tensor.matmul(
                out=ps2[:, 0:D],
                lhsT=mm(hsb[:, 0, :]),
                rhs=mm(w1t[:, 0, :]),
                start=True,
                stop=False,
            )
            nc.tensor.matmul(
                out=ps2[:, 0:D],
                lhsT=mm(hsb[:, 1, :]),
                rhs=mm(w1t[:, 1, :]),
                start=False,
                stop=True,
            )
            whs = whs_pool.tile([P, D], f32, tag="whs")
            nc.vector.tensor_copy(out=whs[:], in_=ps2[:, 0:D])
            nc.sync.dma_start(out=WH1_loc[t * P : (t + 1) * P, :], in_=whs[:])

        gather_pass(nb, bcol, bval, bdl, WHb_full, True, post_bwd)

        nc.gpsimd.collective_compute(
            "AllGather",
            AluOp.bypass,
            replica_groups=rg,
            ins=[WH1_loc[:].opt()],
            outs=[WH1_full[:].opt()],
        )
        if taps:
            nc.sync.dma_start(out=dbg_wh1[:], in_=WH1_full[:])

        # ---------------- pass G: layer-1 forward ------------------------
        def post_fwd1(t, ps):
            h1 = hrm.tile([P, D], f32, tag="h1")
            nc.vector.scalar_tensor_tensor(
                out=h1[:],
                in0=ps[:, 0:D],
                scalar=degc[:, t : t + 1],
                in1=b1bc[:],
                op0=AluOp.mult,
                op1=AluOp.add,
            )
            h2 = hrm.tile([P, D], f32, tag="h2")
            nc.scalar.activation(out=h2[:], in_=h1[:], func=Act.Relu)
            for c in range(DC):
                pt = ps_tr.tile([P, P], f32, tag="tr")
                nc.tensor.transpose(
                    out=pt[:], in_=h2[:, c * P : (c + 1) * P], identity=ident[:]
                )
                nc.vector.tensor_add(x2T[:, c, t, :], pt[:], xT[:, c, t, :])

        gather_pass(n1, fcol, fval, fdl, WH1_full, True, post_fwd1)

        # ---------------- BN-2 + output ----------------------------------
        st2_sb = bn_stats_phase(x2T, tgt_sh, "st2_sb")
        nc.sync.dma_start(out=st2_in[:], in_=st2_sb[:])
        nc.gpsimd.collective_compute(
            "AllReduce",
            AluOp.add,
            replica_groups=rg,
            ins=[st2_in[:].opt()],
            outs=[st2_out[:].opt()],
        )
        st2g = misc.tile([P, 2 * DC], f32, tag="stg")
        nc.sync.dma_start(out=st2g[:], in_=st2_out[:])
        A2, B2 = bn_coeffs(st2g, g2f, be2f, n_tgt, "A2", "B2")

        for t in range(NT):
            for c in range(DC):
                nc.vector.scalar_tensor_tensor(
                    out=x2T[:, c, t, :],
                    in0=x2T[:, c, t, :],
                    scalar=A2[:, c : c + 1],
                    in1=B2[:, c : c + 1].to_broadcast([P, P]),
                    op0=AluOp.mult,
                    op1=AluOp.add,
                )
        for c in range(DC):
            nc.sync.dma_start(
                out=outT[c * P : (c + 1) * P, :],
                in_=x2T[:, c, :, :].rearrange("p a b -> p (a b)"),
            )

    return nc


# ----------------------------------------------------------------- entry


def _run(inputs, trace=False, tmpdir=None, taps=False):
    from concourse.bass_utils import run_bass_kernel_spmd

    H_src = np.asarray(inputs["H_src"], dtype=np.float32)
    target_emb = np.asarray(inputs["target_emb"], dtype=np.float32)
    W_fwd = np.asarray(inputs["W_fwd"], dtype=np.float32)
    b_fwd = np.asarray(inputs["b_fwd"], dtype=np.float32)
    W_bwd = np.asarray(inputs["W_bwd"], dtype=np.float32)
    b_bwd = np.asarray(inputs["b_bwd"], dtype=np.float32)
    gamma = np.asarray(inputs["gamma"], dtype=np.float32)
    beta = np.asarray(inputs["beta"], dtype=np.float32)
    vals = np.asarray(inputs["vals"], dtype=np.float32)
    rows = np.asarray(inputs["rows"])
    cols = np.asarray(inputs["cols"])

    n_src, D = H_src.shape
    n_tgt = target_emb.shape[0]
    assert D == D_FIXED
    tgt_sh = n_tgt // NCORES
    DC = D // P

    n1, f_g, f_v, f_d = _edge_plan(rows, cols, vals, n_tgt, NCORES)
    nb, b_g, b_v, b_d = _edge_plan(cols, rows, vals, n_src, NCORES)

    nc = _build_program(n_tgt, n_src, n1, nb, taps=taps)

    HsrcT = np.ascontiguousarray(H_src.T)
    iota = np.ascontiguousarray(
        np.tile(np.arange(P, dtype=np.float32), (P, 1))
    )

    def fmaj(v):  # [D] -> [P, DC] feature-major
        return np.ascontiguousarray(v.reshape(DC, P).T)

    in_maps = []
    for c in range(NCORES):
        in_maps.append(
            {
                "HsrcT": HsrcT,
                "W0": W_fwd[0],
                "Wb": W_bwd[0],
                "W1": W_fwd[1],
                "b0": b_fwd[0].reshape(1, D),
                "bb": b_bwd[0].reshape(1, D),
                "b1": b_fwd[1].reshape(1, D),
                "g1T": fmaj(gamma[0]),
                "be1T": fmaj(beta[0]),
                "g2T": fmaj(gamma[1]),
                "be2T": fmaj(beta[1]),
                "iota": iota,
                "emb": np.ascontiguousarray(
                    target_emb[c * tgt_sh : (c + 1) * tgt_sh]
                ),
                "fe_col": f_g[c],
                "fe_val": f_v[c],
                "fe_dl": f_d[c],
                "be_row": b_g[c],
                "be_val": b_v[c],
                "be_dl": b_d[c],
            }
        )

    res = run_bass_kernel_spmd(
        nc, in_maps, list(range(NCORES)), trace=trace, tmpdir=tmpdir
    )
    out = np.concatenate(
        [res.results[c]["outT"].T for c in range(NCORES)], axis=0
    )
    return np.ascontiguousarray(out), res


def kernel(**inputs) -> np.ndarray:
    out, _ = _run(inputs)
    return out


# revision 22
# speedup vs baseline: 1.1700x; 1.1700x over previous
"""Bipartite GCN stack (2 layers) on 8 Trainium2 NeuronCores.

Strategy (graph/data parallel, destination-sharded):
  - Layer-0 forward: every core computes the full WH0 = H_src @ W_fwd0
    (redundantly; cheaper than all-gathering the 64MB table), then
    processes the ~1/8 of edges whose destination (row) falls in its
    4096-target shard.  segment_sum is done by gathering WH0 rows in
    1024-row batches with the Q7 dma_gather instruction and reducing
    each 128-edge tile into PSUM with a selection-matrix matmul
    (S[e,d] = val[e] if dst_local[e]==d).  A second tiny matmul against
    a ones column accumulates the degrees.
  - dma_gather needs int16 indices, so 65536-row tables are split into
    lo/hi halves of 32768 rows and each destination tile's edges are
    grouped by half (host-side index preprocessing only).
  - BatchNorm: per-core partial sums via bn_stats in feature-major
    layout, 2KB AllReduce, scale/shift applied in feature-major form
    (which is exactly the lhsT layout the next dense matmul needs).
  - Backward (layer 0 only; the layer-1 backward output is dead):
    AllGather of WHb, edges sharded by source, same reduction.
  - Layer-1 forward: AllGather of WH1, reuse of the layer-0 target
    degrees, residual in feature-major form, BN, output.

Gathered tables are stored in _TABLE_DT (bfloat16 by default: 4x PE
matmul rate, half the gather/collective bytes); everything else is
fp32.  Host-side work is limited to sharding/permutation of the edge
index arrays and layout transforms of inputs/outputs; all
floating-point math runs on the NeuronCores.
"""

import numpy as np

P = 128
D_FIXED = 256
EPS = 1e-5
NCORES = 8
GBT = 8  # gather batch: edge tiles per dma_gather (1024 rows)

# dtype of the gathered feature tables (WH0/WHb/WH1), the gather buffers
# and the selection matrices: "float32" (exact) or "bfloat16"
_TABLE_DT = "bfloat16"


# ----------------------------------------------------------------- host prep


def _edge_plan(dst, gidx, vals, n_dst, n_gather, ncores, split):
    """Partition edges by destination shard, group by 128-row dst tile and
    (optionally) by gather-table half; pad each (core, tile[, half]) group
    to a multiple of 128 edges, common across cores.

    Returns:
      segs:   per dst-tile, list of (half, ntiles)
      idx16:  per-core [128, NF*8] int16 gather indices (16-partition
              wrapped layout for dma_gather, replicated 8x vertically)
      val:    per-core [P, NF] f32
      dl:     per-core [P, NF] f32 (dst_local in 0..127)
    """
    dst_sh = n_dst // ncores
    nt = dst_sh // P
    nhalf = 2 if split else 1
    half_rows = n_gather // nhalf

    core_of = dst // dst_sh
    tile_of = (dst % dst_sh) // P
    dl_of = (dst % P).astype(np.float32)
    half_of = (gidx // half_rows) if split else np.zeros(len(dst), np.int64)
    lidx = (gidx - half_of * half_rows).astype(np.int16)

    grp = (core_of * nt + tile_of) * nhalf + half_of
    order = np.argsort(grp, kind="stable")
    so_lidx = lidx[order]
    so_val = vals[order].astype(np.float32)
    so_dl = dl_of[order]

    ngrp = ncores * nt * nhalf
    counts = np.bincount(grp, minlength=ngrp).reshape(ncores, nt, nhalf)
    # per (tile, half) tile count, common across cores; >=1 tile per dst tile
    ntile_th = np.ceil(counts.max(axis=0) / P).astype(np.int64)  # [nt, nhalf]
    for t in range(nt):
        if ntile_th[t].sum() == 0:
            ntile_th[t, 0] = 1
    nf = int(ntile_th.sum())
    off_flat = np.concatenate([[0], np.cumsum(ntile_th.reshape(-1))])

    i_arr = np.zeros((ncores, nf * P), dtype=np.int16)
    v_arr = np.zeros((ncores, nf * P), dtype=np.float32)
    d_arr = np.zeros((ncores, nf * P), dtype=np.float32)

    grp_start = np.concatenate([[0], np.cumsum(counts.reshape(-1))])
    for c in range(ncores):
        for t in range(nt):
            for h in range(nhalf):
                gi = (c * nt + t) * nhalf + h
                s, e = grp_start[gi], grp_start[gi + 1]
                n = e - s
                o = off_flat[t * nhalf + h] * P
                i_arr[c, o : o + n] = so_lidx[s:e]
                v_arr[c, o : o + n] = so_val[s:e]
                d_arr[c, o : o + n] = so_dl[s:e]

    segs = [
        [(h, int(ntile_th[t, h])) for h in range(nhalf) if ntile_th[t, h] > 0]
        for t in range(nt)
    ]
    # dma_gather index layout: linear idx i at [i % 16, i // 16], x8 vertical
    idx16 = []
    for c in range(ncores):
        a = i_arr[c].reshape(nf * 8, 16).T  # [16, nf*8]
        idx16.append(np.ascontiguousarray(np.tile(a, (8, 1))))
    v_dev = [np.ascontiguousarray(v_arr[c].reshape(nf, P).T) for c in range(ncores)]
    d_dev = [np.ascontiguousarray(d_arr[c].reshape(nf, P).T) for c in range(ncores)]
    return segs, idx16, v_dev, d_dev


# ----------------------------------------------------------------- bass build


def _install_drain_patch():
    """walrus in this env allows only ONE sem-wait per instruction; split
    extra waits onto same-engine carrier instructions."""
    import concourse.mybir as mybir
    import concourse.tile as _tile
    from concourse.vector_clock import ScopedClock

    if getattr(_tile.TileContext, "_drain_split_patched", False):
        return

    def _split_drain_and_barrier(self, tick_clock, wait_clock):
        nc = self.nc
        drain_inst = nc.sync.drain()
        wait_clock.add_sem_waits(
            drain_inst.ins, ScopedClock({None: tick_clock.global_clock})
        )
        si = drain_inst.ins.sync_info
        waits = list(si.on_wait) if si and si.on_wait else []
        if len(waits) > 1:
            si.on_wait = waits[:1]
            drain_inst.ins.sync_info = si
            for i in range(1, len(waits)):
                extra = nc.sync.drain()
                esi = extra.ins.sync_info
                upd = list(esi.on_update) if esi and esi.on_update else []
                extra.ins.sync_info = mybir.SyncInfo(
                    on_wait=[waits[i]], on_update=upd
                )
        nc.all_engine_barrier()
        assert self.sems is not None
        popped = nc._tile_sem_poison_stack.pop()
        assert popped is self._sem_poison
        nc.clear_and_free_semaphores(list(self.sems.allocated().values()))
        nc.all_engine_barrier()

    _tile.TileContext._drain_and_barrier = _split_drain_and_barrier

    _orig_add = _tile.TileContext._add_instruction

    def _add_instruction_split(self, inst):
        si = inst.sync_info
        waits = list(si.on_wait) if si and si.on_wait else []
        if len(waits) > 1 and inst.engine != mybir.EngineType.Unassigned:
            for w in waits[:-1]:
                nop = mybir.InstNoOp(
                    name=self.nc.get_next_instruction_name(), ins=[], outs=[]
                )
                nop.engine = inst.engine
                nop.sync_info = mybir.SyncInfo(on_wait=[w], on_update=[])
                _orig_add(self, nop)
            si.on_wait = waits[-1:]
            inst.sync_info = si
        _orig_add(self, inst)

    _tile.TileContext._add_instruction = _add_instruction_split
    _tile.TileContext._drain_split_patched = True


def _build_program(n_tgt, n_src, fsegs, bsegs, taps=False):
    """Build the SPMD bass program (identical on all 8 cores)."""
    from contextlib import ExitStack

    import concourse.bass as bass
    import concourse.mybir as mybir
    import concourse.tile as tile
    from concourse import bacc
    from concourse.masks import make_identity

    _install_drain_patch()

    dt = mybir.dt
    f32 = dt.float32
    i16 = dt.int16
    tb = getattr(dt, _TABLE_DT)
    D = D_FIXED
    DC = D // P
    tgt_sh = n_tgt // NCORES
    src_sh = n_src // NCORES
    NT = tgt_sh // P
    NF = sum(n for seg in fsegs for _, n in seg)
    NB = sum(n for seg in bsegs for _, n in seg)
    AluOp = mybir.AluOpType
    Act = mybir.ActivationFunctionType
    rg = [list(range(NCORES))]

    nc = bacc.Bacc("TRN2", target_bir_lowering=False, debug=False, num_devices=NCORES)

    dram_t = nc.dram_tensor
    HsrcT = dram_t("HsrcT", [D, n_src], f32, kind="ExternalInput").ap()
    W0 = dram_t("W0", [D, D], f32, kind="ExternalInput").ap()
    Wb = dram_t("Wb", [D, D], f32, kind="ExternalInput").ap()
    W1 = dram_t("W1", [D, D], f32, kind="ExternalInput").ap()
    b0_h = dram_t("b0", [1, D], f32, kind="ExternalInput")
    bb_h = dram_t("bb", [1, D], f32, kind="ExternalInput")
    b1_h = dram_t("b1", [1, D], f32, kind="ExternalInput")
    g1T = dram_t("g1T", [P, DC], f32, kind="ExternalInput").ap()
    be1T = dram_t("be1T", [P, DC], f32, kind="ExternalInput").ap()
    g2T = dram_t("g2T", [P, DC], f32, kind="ExternalInput").ap()
    be2T = dram_t("be2T", [P, DC], f32, kind="ExternalInput").ap()
    iota_d = dram_t("iota", [P, P], f32, kind="ExternalInput").ap()
    emb = dram_t("emb", [tgt_sh, D], f32, kind="ExternalInput").ap()
    fe_i16 = dram_t("fe_i16", [P, NF * 8], i16, kind="ExternalInput").ap()
    fe_val = dram_t("fe_val", [P, NF], f32, kind="ExternalInput").ap()
    fe_dl = dram_t("fe_dl", [P, NF], f32, kind="ExternalInput").ap()
    be_i16 = dram_t("be_i16", [P, NB * 8], i16, kind="ExternalInput").ap()
    be_val = dram_t("be_val", [P, NB], f32, kind="ExternalInput").ap()
    be_dl = dram_t("be_dl", [P, NB], f32, kind="ExternalInput").ap()
    outT = dram_t("outT", [D, tgt_sh], f32, kind="ExternalOutput").ap()
    if taps:
        dbg_wh0 = dram_t("dbg_wh0", [n_src, D], tb, kind="ExternalOutput").ap()
        dbg_x1T = dram_t("dbg_x1T", [D, tgt_sh], f32, kind="ExternalOutput").ap()
        dbg_whb = dram_t("dbg_whb", [n_tgt, D], tb, kind="ExternalOutput").ap()
        dbg_wh1 = dram_t("dbg_wh1", [n_src, D], tb, kind="ExternalOutput").ap()
        dbg_st1 = dram_t("dbg_st1", [P, 4], f32, kind="ExternalOutput").ap()
        dbg_x1pre = dram_t("dbg_x1pre", [D, tgt_sh], f32, kind="ExternalOutput").ap()
        dbg_deg = dram_t("dbg_deg", [P, NT], f32, kind="ExternalOutput").ap()

    with tile.TileContext(nc) as tc, ExitStack() as ctx:
        dram = ctx.enter_context(tc.tile_pool(name="dram", bufs=1, space="DRAM"))
        half_rows = n_src // 2
        WH0_t = [dram.tile([half_rows, D], tb, name=f"WH0h{h}") for h in range(2)]
        WHb_loc = dram.tile([tgt_sh, D], tb)
        WHb_full = dram.tile([n_tgt, D], tb, addr_space="Shared")
        WH1_loc = dram.tile([src_sh, D], tb)
        WH1_full = dram.tile([n_src, D], tb, addr_space="Shared")
        WH1_t = [
            WH1_full[h * half_rows : (h + 1) * half_rows, :] for h in range(2)
        ]
        st1_in = dram.tile([P, 2 * DC], f32)
        st1_out = dram.tile([P, 2 * DC], f32, addr_space="Shared")
        st2_in = dram.tile([P, 2 * DC], f32)
        st2_out = dram.tile([P, 2 * DC], f32, addr_space="Shared")

        consts = ctx.enter_context(tc.tile_pool(name="consts", bufs=1))
        w0t = consts.tile([P, DC, D], f32)
        wbt = consts.tile([P, DC, D], f32)
        w1t = consts.tile([P, DC, D], f32)
        for c in range(DC):
            nc.sync.dma_start(out=w0t[:, c, :], in_=W0[c * P : (c + 1) * P, :])
            nc.sync.dma_start(out=wbt[:, c, :], in_=Wb[c * P : (c + 1) * P, :])
            nc.sync.dma_start(out=w1t[:, c, :], in_=W1[c * P : (c + 1) * P, :])
        w0b = consts.tile([P, DC, D], tb)
        wbb = consts.tile([P, DC, D], tb)
        for c in range(DC):
            nc.vector.tensor_copy(out=w0b[:, c, :], in_=w0t[:, c, :])
            nc.vector.tensor_copy(out=wbb[:, c, :], in_=wbt[:, c, :])
        b0bc = consts.tile([P, D], f32)
        bbbc = consts.tile([P, D], f32)
        b1bc = consts.tile([P, D], f32)
        for h_, t_ in ((b0_h, b0bc), (bb_h, bbbc), (b1_h, b1bc)):
            nc.gpsimd.dma_start(
                out=t_[:], in_=bass.AP(tensor=h_, offset=0, ap=[[0, P], [1, D]])
            )
        g1f = consts.tile([P, DC], f32)
        be1f = consts.tile([P, DC], f32)
        g2f = consts.tile([P, DC], f32)
        be2f = consts.tile([P, DC], f32)
        nc.sync.dma_start(out=g1f[:], in_=g1T[:])
        nc.sync.dma_start(out=be1f[:], in_=be1T[:])
        nc.sync.dma_start(out=g2f[:], in_=g2T[:])
        nc.sync.dma_start(out=be2f[:], in_=be2T[:])
        iota_t = consts.tile([P, P], f32)
        nc.sync.dma_start(out=iota_t[:], in_=iota_d[:])
        epst = consts.tile([P, 1], f32)
        nc.vector.memset(epst[:], EPS)
        ident = consts.tile([P, P], f32)
        make_identity(nc, ident[:])
        onesb = consts.tile([P, 1], tb)
        nc.vector.memset(onesb[:], 1.0)

        # resident state
        xT = consts.tile([P, DC, NT, P], tb)  # H_tgt1 (feature-major)
        x2T = consts.tile([P, DC, NT, P], f32)  # layer-1 pre-BN x
        degc = consts.tile([P, NT], f32)  # reciprocal clamped target degree

        sv_pool = ctx.enter_context(tc.tile_pool(name="sv", bufs=8))
        g_pool = ctx.enter_context(tc.tile_pool(name="gp", bufs=4))
        ps_acc = ctx.enter_context(tc.tile_pool(name="psacc", bufs=3, space="PSUM"))
        ps_tr = ctx.enter_context(tc.tile_pool(name="pstr", bufs=2, space="PSUM"))
        hrm = ctx.enter_context(tc.tile_pool(name="hrm", bufs=4))
        whs_pool = ctx.enter_context(tc.tile_pool(name="whs", bufs=4))
        lhs_pool = ctx.enter_context(tc.tile_pool(name="lhs", bufs=2))
        misc = ctx.enter_context(tc.tile_pool(name="misc", bufs=6))

        # ---------------- phase A: full WH0 = H_src @ W0 (every core) ----
        SUP = 512 if n_src % 512 == 0 else P
        for st in range(n_src // SUP):
            haf = lhs_pool.tile([P, SUP], f32, tag="haf")
            hbf = lhs_pool.tile([P, SUP], f32, tag="hbf")
            nc.sync.dma_start(out=haf[:], in_=HsrcT[0:P, st * SUP : (st + 1) * SUP])
            nc.sync.dma_start(
                out=hbf[:], in_=HsrcT[P : 2 * P, st * SUP : (st + 1) * SUP]
            )
            ha = lhs_pool.tile([P, SUP], tb, tag="ha")
            hb = lhs_pool.tile([P, SUP], tb, tag="hb")
            nc.vector.tensor_copy(out=ha[:], in_=haf[:])
            nc.vector.tensor_copy(out=hb[:], in_=hbf[:])
            for r in range(SUP // P):
                ps = ps_acc.tile([P, D], f32, tag="acc")
                nc.tensor.matmul(
                    out=ps[:],
                    lhsT=ha[:, r * P : (r + 1) * P],
                    rhs=w0b[:, 0, :],
                    start=True,
                    stop=False,
                )
                nc.tensor.matmul(
                    out=ps[:],
                    lhsT=hb[:, r * P : (r + 1) * P],
                    rhs=w0b[:, 1, :],
                    start=False,
                    stop=True,
                )
                whs = whs_pool.tile([P, D], tb, tag="whs")
                nc.vector.tensor_copy(out=whs[:], in_=ps[:])
                row0 = (st * (SUP // P) + r) * P
                h = row0 // half_rows
                lr = row0 - h * half_rows
                nc.sync.dma_start(out=WH0_t[h][lr : lr + P, :], in_=whs[:])

        # ---------------- gather pass helper ----------------------------
        def gather_pass(segs, idx_tile, val_tile, dl_tile, tables, want_deg, post):
            """segs: per dst-tile list of (table_idx, ntiles)."""
            j = 0  # global edge-tile index
            for t in range(len(segs)):
                ntile_tot = sum(n for _, n in segs[t])
                ps = ps_acc.tile([P, D], f32, tag="acc", name=f"ps{t}")
                psd = None
                if want_deg:
                    psd = ps_acc.tile([P, 1], f32, tag="deg", bufs=2, name=f"psd{t}")
                k = 0  # tile index within dst-tile
                for tab_i, nseg in segs[t]:
                    table = tables[tab_i]
                    done = 0
                    while done < nseg:
                        bs = min(GBT, nseg - done)
                        gt = g_pool.tile([P, GBT, D], tb, tag="gt", name="gt")
                        nc.gpsimd.dma_gather(
                            out_ap=gt[:, 0:bs, :],
                            in_ap=table,
                            idxs_ap=idx_tile[:, j * 8 : (j + bs) * 8],
                            num_idxs=bs * P,
                            num_idxs_reg=bs * P,
                            elem_size=D,
                        )
                        for i in range(bs):
                            jj = j + i
                            svf = sv_pool.tile([P, P], f32, name="svf", tag="svf")
                            nc.vector.scalar_tensor_tensor(
                                out=svf[:],
                                in0=iota_t[:],
                                scalar=dl_tile[:, jj : jj + 1],
                                in1=val_tile[:, jj : jj + 1].to_broadcast([P, P]),
                                op0=AluOp.is_equal,
                                op1=AluOp.mult,
                            )
                            sv = sv_pool.tile([P, P], tb, name="sv")
                            nc.vector.tensor_copy(out=sv[:], in_=svf[:])
                            nc.tensor.matmul(
                                out=ps[:],
                                lhsT=sv[:],
                                rhs=gt[:, i, :],
                                start=(k + i == 0),
                                stop=(k + i == ntile_tot - 1),
                            )
                            if psd is not None:
                                nc.tensor.matmul(
                                    out=psd[:],
                                    lhsT=sv[:],
                                    rhs=onesb[:],
                                    start=(k + i == 0),
                                    stop=(k + i == ntile_tot - 1),
                                )
                        j += bs
                        done += bs
                        k += bs
                post(t, ps, psd)
            return j

        # ---------------- pass B: layer-0 forward ------------------------
        def post_fwd0(t, ps, psd):
            dtmp = misc.tile([P, 1], f32, tag="dtgt")
            nc.vector.tensor_scalar_max(dtmp[:], psd[:], 1.0)
            nc.vector.reciprocal(degc[:, t : t + 1], dtmp[:])
            h1 = hrm.tile([P, D], f32, tag="h1")
            nc.vector.scalar_tensor_tensor(
                out=h1[:],
                in0=ps[:],
                scalar=degc[:, t : t + 1],
                in1=b0bc[:],
                op0=AluOp.mult,
                op1=AluOp.add,
            )
            h2 = hrm.tile([P, D], f32, tag="h2")
            nc.scalar.activation(out=h2[:], in_=h1[:], func=Act.Relu)
            et = misc.tile([P, D], f32, tag="emb")
            nc.sync.dma_start(out=et[:], in_=emb[t * P : (t + 1) * P, :])
            xr = hrm.tile([P, D], f32, tag="xr")
            nc.vector.tensor_add(xr[:], h2[:], et[:])
            for c in range(DC):
                pt = ps_tr.tile([P, P], f32, tag="tr")
                nc.tensor.transpose(
                    out=pt[:], in_=xr[:, c * P : (c + 1) * P], identity=ident[:]
                )
                nc.vector.tensor_copy(out=xT[:, c, t, :], in_=pt[:])

        with tc.tile_pool(name="edgesB", bufs=1) as ep:
            fidx = ep.tile([P, NF * 8], i16, name="fidxB")
            fval = ep.tile([P, NF], f32, name="fvalB")
            fdl = ep.tile([P, NF], f32, name="fdlB")
            nc.sync.dma_start(out=fidx[:], in_=fe_i16[:])
            nc.sync.dma_start(out=fval[:], in_=fe_val[:])
            nc.sync.dma_start(out=fdl[:], in_=fe_dl[:])
            gather_pass(
                fsegs, fidx, fval, fdl, [t_[:] for t_ in WH0_t], True, post_fwd0
            )

        # ---------------- BN helpers -------------------------------------
        def bn_stats_phase(xbuf, count, st_in_sb_name):
            st_sb = misc.tile([P, 2 * DC], f32, name=st_in_sb_name, tag="stats")
            grp = min(512, count)
            ngrp = count // grp
            for c in range(DC):
                bnst = misc.tile([P, ngrp, 6], f32, tag="bnst")
                flat = xbuf[:, c, :, :].rearrange("p a b -> p (a b)")
                for g in range(ngrp):
                    nc.vector.bn_stats(
                        out=bnst[:, g, :], in_=flat[:, g * grp : (g + 1) * grp]
                    )
                mv = misc.tile([P, 2], f32, tag="mv")
                nc.vector.bn_aggr(out=mv[:], in_=bnst[:].rearrange("p a b -> p (a b)"))
                nc.vector.tensor_scalar_mul(
                    st_sb[:, 2 * c : 2 * c + 1], mv[:, 0:1], float(count)
                )
                musq = misc.tile([P, 1], f32, tag="musq")
                nc.vector.tensor_mul(musq[:], mv[:, 0:1], mv[:, 0:1])
                nc.vector.tensor_add(musq[:], musq[:], mv[:, 1:2])
                nc.vector.tensor_scalar_mul(
                    st_sb[:, 2 * c + 1 : 2 * c + 2], musq[:], float(count)
                )
            return st_sb

        def bn_coeffs(st_full_sb, gamma_f, beta_f, total, a_name, b_name):
            A = misc.tile([P, DC], f32, name=a_name, tag="bnA")
            B = misc.tile([P, DC], f32, name=b_name, tag="bnB")
            for c in range(DC):
                mu = misc.tile([P, 1], f32, tag="mu")
                nc.vector.tensor_scalar_mul(
                    mu[:], st_full_sb[:, 2 * c : 2 * c + 1], 1.0 / total
                )
                q = misc.tile([P, 1], f32, tag="q")
                nc.vector.tensor_scalar_mul(
                    q[:], st_full_sb[:, 2 * c + 1 : 2 * c + 2], 1.0 / total
                )
                musq = misc.tile([P, 1], f32, tag="musq2")
                nc.vector.tensor_mul(musq[:], mu[:], mu[:])
                var = misc.tile([P, 1], f32, tag="var")
                nc.vector.tensor_tensor(
                    out=var[:], in0=q[:], in1=musq[:], op=AluOp.subtract
                )
                sd = misc.tile([P, 1], f32, tag="sd")
                nc.scalar.activation(out=sd[:], in_=var[:], func=Act.Sqrt, bias=epst[:])
                rstd = misc.tile([P, 1], f32, tag="rstd")
                nc.vector.reciprocal(rstd[:], sd[:])
                nc.vector.tensor_mul(A[:, c : c + 1], gamma_f[:, c : c + 1], rstd[:])
                mA = misc.tile([P, 1], f32, tag="mA")
                nc.vector.tensor_mul(mA[:], mu[:], A[:, c : c + 1])
                nc.vector.tensor_tensor(
                    out=B[:, c : c + 1],
                    in0=beta_f[:, c : c + 1],
                    in1=mA[:],
                    op=AluOp.subtract,
                )
            return A, B

        if taps:
            for h in range(2):
                nc.sync.dma_start(
                    out=dbg_wh0[h * half_rows : (h + 1) * half_rows, :],
                    in_=WH0_t[h][:],
                )
            nc.sync.dma_start(out=dbg_deg[:], in_=degc[:])
            for c in range(DC):
                nc.sync.dma_start(
                    out=dbg_x1pre[c * P : (c + 1) * P, :],
                    in_=xT[:, c, :, :].rearrange("p a b -> p (a b)"),
                )

        # ---------------- BN-1 + WHb + AllGather -------------------------
        st1_sb = bn_stats_phase(xT, tgt_sh, "st1_sb")
        nc.sync.dma_start(out=st1_in[:], in_=st1_sb[:])
        nc.gpsimd.collective_compute(
            "AllReduce",
            AluOp.add,
            replica_groups=rg,
            ins=[st1_in[:].opt()],
            outs=[st1_out[:].opt()],
        )
        st1g = misc.tile([P, 2 * DC], f32, tag="stg")
        nc.sync.dma_start(out=st1g[:], in_=st1_out[:])
        A1, B1 = bn_coeffs(st1g, g1f, be1f, n_tgt, "A1", "B1")

        for t in range(NT):
            for c in range(DC):
                nc.vector.scalar_tensor_tensor(
                    out=xT[:, c, t, :],
                    in0=xT[:, c, t, :],
                    scalar=A1[:, c : c + 1],
                    in1=B1[:, c : c + 1].to_broadcast([P, P]),
                    op0=AluOp.mult,
                    op1=AluOp.add,
                )
            ps = ps_acc.tile([P, D], f32, tag="acc", name=f"pswb{t}")
            nc.tensor.matmul(
                out=ps[:], lhsT=xT[:, 0, t, :], rhs=wbb[:, 0, :], start=True, stop=False
            )
            nc.tensor.matmul(
                out=ps[:], lhsT=xT[:, 1, t, :], rhs=wbb[:, 1, :], start=False, stop=True
            )
            whs = whs_pool.tile([P, D], tb, tag="whs")
            nc.vector.tensor_copy(out=whs[:], in_=ps[:])
            nc.sync.dma_start(out=WHb_loc[t * P : (t + 1) * P, :], in_=whs[:])

        nc.gpsimd.collective_compute(
            "AllGather",
            AluOp.bypass,
            replica_groups=rg,
            ins=[WHb_loc[:].opt()],
            outs=[WHb_full[:].opt()],
        )
        if taps:
            nc.sync.dma_start(out=dbg_st1[:], in_=st1_out[:])
            for c in range(DC):
                nc.sync.dma_start(
                    out=dbg_x1T[c * P : (c + 1) * P, :],
                    in_=xT[:, c, :, :].rearrange("p a b -> p (a b)"),
                )
            nc.sync.dma_start(out=dbg_whb[:], in_=WHb_full[:])

        # ---------------- pass E: layer-0 backward (+ fused WH1) ---------
        def post_bwd(t, ps, psd):
            dtmp = misc.tile([P, 1], f32, tag="dsrc")
            nc.vector.tensor_scalar_max(dtmp[:], psd[:], 1.0)
            rtmp = misc.tile([P, 1], f32, tag="rsrc")
            nc.vector.reciprocal(rtmp[:], dtmp[:])
            h1 = hrm.tile([P, D], f32, tag="h1")
            nc.vector.scalar_tensor_tensor(
                out=h1[:],
                in0=ps[:],
                scalar=rtmp[:],
                in1=bbbc[:],
                op0=AluOp.mult,
                op1=AluOp.add,
            )
            h2 = hrm.tile([P, D], f32, tag="h2")
            nc.scalar.activation(out=h2[:], in_=h1[:], func=Act.Relu)
            hsb = misc.tile([P, DC, P], f32, tag="hsb")
            for c in range(DC):
                pt = ps_tr.tile([P, P], f32, tag="tr")
                nc.tensor.transpose(
                    out=pt[:], in_=h2[:, c * P : (c + 1) * P], identity=ident[:]
                )
                nc.vector.tensor_copy(out=hsb[:, c, :], in_=pt[:])
            ps2 = ps_acc.tile([P, D], f32, tag="acc", name=f"psw1{t}")
            nc.tensor.matmul(
                out=ps2[:], lhsT=hsb[:, 0, :], rhs=w1t[:, 0, :], start=True, stop=False
            )
            nc.tensor.matmul(
                out=ps2[:], lhsT=hsb[:, 1, :], rhs=w1t[:, 1, :], start=False, stop=True
            )
            whs = whs_pool.tile([P, D], tb, tag="whs")
            nc.vector.tensor_copy(out=whs[:], in_=ps2[:])
            nc.sync.dma_start(out=WH1_loc[t * P : (t + 1) * P, :], in_=whs[:])

        with tc.tile_pool(name="edgesE", bufs=1) as ep:
            bidx = ep.tile([P, NB * 8], i16, name="bidxE")
            bval = ep.tile([P, NB], f32, name="bvalE")
            bdl = ep.tile([P, NB], f32, name="bdlE")
            nc.sync.dma_start(out=bidx[:], in_=be_i16[:])
            nc.sync.dma_start(out=bval[:], in_=be_val[:])
            nc.sync.dma_start(out=bdl[:], in_=be_dl[:])
            gather_pass(bsegs, bidx, bval, bdl, [WHb_full[:]], True, post_bwd)

        nc.gpsimd.collective_compute(
            "AllGather",
            AluOp.bypass,
            replica_groups=rg,
            ins=[WH1_loc[:].opt()],
            outs=[WH1_full[:].opt()],
        )
        if taps:
            nc.sync.dma_start(out=dbg_wh1[:], in_=WH1_full[:])

        # ---------------- pass G: layer-1 forward ------------------------
        def post_fwd1(t, ps, psd):
            h1 = hrm.tile([P, D], f32, tag="h1")
            nc.vector.scalar_tensor_tensor(
                out=h1[:],
                in0=ps[:],
                scalar=degc[:, t : t + 1],
                in1=b1bc[:],
                op0=AluOp.mult,
                op1=AluOp.add,
            )
            h2 = hrm.tile([P, D], f32, tag="h2")
            nc.scalar.activation(out=h2[:], in_=h1[:], func=Act.Relu)
            for c in range(DC):
                pt = ps_tr.tile([P, P], f32, tag="tr")
                nc.tensor.transpose(
                    out=pt[:], in_=h2[:, c * P : (c + 1) * P], identity=ident[:]
                )
                nc.vector.tensor_add(x2T[:, c, t, :], pt[:], xT[:, c, t, :])

        with tc.tile_pool(name="edgesG", bufs=1) as ep:
            fidx2 = ep.tile([P, NF * 8], i16, name="fidxG")
            fval2 = ep.tile([P, NF], f32, name="fvalG")
            fdl2 = ep.tile([P, NF], f32, name="fdlG")
            nc.sync.dma_start(out=fidx2[:], in_=fe_i16[:])
            nc.sync.dma_start(out=fval2[:], in_=fe_val[:])
            nc.sync.dma_start(out=fdl2[:], in_=fe_dl[:])
            gather_pass(
                fsegs, fidx2, fval2, fdl2, [t_[:] for t_ in WH1_t], False, post_fwd1
            )

        # ---------------- BN-2 + output ----------------------------------
        st2_sb = bn_stats_phase(x2T, tgt_sh, "st2_sb")
        nc.sync.dma_start(out=st2_in[:], in_=st2_sb[:])
        nc.gpsimd.collective_compute(
            "AllReduce",
            AluOp.add,
            replica_groups=rg,
            ins=[st2_in[:].opt()],
            outs=[st2_out[:].opt()],
        )
        st2g = misc.tile([P, 2 * DC], f32, tag="stg")
        nc.sync.dma_start(out=st2g[:], in_=st2_out[:])
        A2, B2 = bn_coeffs(st2g, g2f, be2f, n_tgt, "A2", "B2")

        for t in range(NT):
            for c in range(DC):
                nc.vector.scalar_tensor_tensor(
                    out=x2T[:, c, t, :],
                    in0=x2T[:, c, t, :],
                    scalar=A2[:, c : c + 1],
                    in1=B2[:, c : c + 1].to_broadcast([P, P]),
                    op0=AluOp.mult,
                    op1=AluOp.add,
                )
        for c in range(DC):
            nc.sync.dma_start(
                out=outT[c * P : (c + 1) * P, :],
                in_=x2T[:, c, :, :].rearrange("p a b -> p (a b)"),
            )

    nc.compile()
    return nc


# ----------------------------------------------------------------- entry


def _run(inputs, trace=False, tmpdir=None, taps=False):
    from concourse.bass_utils import run_bass_kernel_spmd

    H_src = np.asarray(inputs["H_src"], dtype=np.float32)
    target_emb = np.asarray(inputs["target_emb"], dtype=np.float32)
    W_fwd = np.asarray(inputs["W_fwd"], dtype=np.float32)
    b_fwd = np.asarray(inputs["b_fwd"], dtype=np.float32)
    W_bwd = np.asarray(inputs["W_bwd"], dtype=np.float32)
    b_bwd = np.asarray(inputs["b_bwd"], dtype=np.float32)
    gamma = np.asarray(inputs["gamma"], dtype=np.float32)
    beta = np.asarray(inputs["beta"], dtype=np.float32)
    vals = np.asarray(inputs["vals"], dtype=np.float32)
    rows = np.asarray(inputs["rows"])
    cols = np.asarray(inputs["cols"])

    n_src, D = H_src.shape
    n_tgt = target_emb.shape[0]
    assert D == D_FIXED
    tgt_sh = n_tgt // NCORES
    DC = D // P

    fsegs, f_i, f_v, f_d = _edge_plan(
        rows, cols, vals, n_tgt, n_src, NCORES, split=True
    )
    bsegs, b_i, b_v, b_d = _edge_plan(
        cols, rows, vals, n_src, n_tgt, NCORES, split=False
    )

    nc = _build_program(n_tgt, n_src, fsegs, bsegs, taps=taps)

    HsrcT = np.ascontiguousarray(H_src.T)
    iota = np.ascontiguousarray(np.tile(np.arange(P, dtype=np.float32), (P, 1)))

    def fmaj(v):  # [D] -> [P, DC] feature-major
        return np.ascontiguousarray(v.reshape(DC, P).T)

    in_maps = []
    for c in range(NCORES):
        in_maps.append(
            {
                "HsrcT": HsrcT,
                "W0": W_fwd[0],
                "Wb": W_bwd[0],
                "W1": W_fwd[1],
                "b0": b_fwd[0].reshape(1, D),
                "bb": b_bwd[0].reshape(1, D),
                "b1": b_fwd[1].reshape(1, D),
                "g1T": fmaj(gamma[0]),
                "be1T": fmaj(beta[0]),
                "g2T": fmaj(gamma[1]),
                "be2T": fmaj(beta[1]),
                "iota": iota,
                "emb": np.ascontiguousarray(
                    target_emb[c * tgt_sh : (c + 1) * tgt_sh]
                ),
                "fe_i16": f_i[c],
                "fe_val": f_v[c],
                "fe_dl": f_d[c],
                "be_i16": b_i[c],
                "be_val": b_v[c],
                "be_dl": b_d[c],
            }
        )

    res = run_bass_kernel_spmd(
        nc, in_maps, list(range(NCORES)), trace=trace, tmpdir=tmpdir
    )
    out = np.concatenate(
        [np.asarray(res.results[c]["outT"]).astype(np.float32).T for c in range(NCORES)],
        axis=0,
    )
    return np.ascontiguousarray(out), res


def kernel(**inputs) -> np.ndarray:
    out, _ = _run(inputs)
    return out


# revision 24
# speedup vs baseline: 1.2319x; 1.0529x over previous
"""Bipartite GCN stack (2 layers) on 8 Trainium2 NeuronCores.

Strategy (graph/data parallel, destination-sharded):
  - Layer-0 forward: every core computes the full WH0 = H_src @ W_fwd0
    (redundantly; cheaper than all-gathering the 64MB table), then
    processes the ~1/8 of edges whose destination (row) falls in its
    4096-target shard.  segment_sum is done by gathering WH0 rows in
    1024-row batches with the Q7 dma_gather instruction and reducing
    each 128-edge tile into PSUM with a selection-matrix matmul
    (S[e,d] = val[e] if dst_local[e]==d).  A second tiny matmul against
    a ones column accumulates the degrees.
  - dma_gather needs int16 indices, so 65536-row tables are split into
    lo/hi halves of 32768 rows and each destination tile's edges are
    grouped by half (host-side index preprocessing only).
  - BatchNorm: per-core partial sums via bn_stats in feature-major
    layout, 2KB AllReduce, scale/shift applied in feature-major form
    (which is exactly the lhsT layout the next dense matmul needs).
  - Backward (layer 0 only; the layer-1 backward output is dead):
    AllGather of WHb, edges sharded by source, same reduction.
  - Layer-1 forward: AllGather of WH1, reuse of the layer-0 target
    degrees, residual in feature-major form, BN, output.

Gathered tables are stored in _TABLE_DT (bfloat16 by default: 4x PE
matmul rate, half the gather/collective bytes); everything else is
fp32.  Host-side work is limited to sharding/permutation of the edge
index arrays and layout transforms of inputs/outputs; all
floating-point math runs on the NeuronCores.
"""

import numpy as np

P = 128
D_FIXED = 256
EPS = 1e-5
NCORES = 8
GBT = 8  # gather batch: edge tiles per dma_gather (1024 rows)

# dtype of the gathered feature tables (WH0/WHb/WH1), the gather buffers
# and the selection matrices: "float32" (exact) or "bfloat16"
_TABLE_DT = "bfloat16"


# ----------------------------------------------------------------- host prep


def _edge_plan(dst, gidx, vals, n_dst, n_gather, ncores, split):
    """Partition edges by destination shard, group by 128-row dst tile and
    (optionally) by gather-table half; pad each (core, tile[, half]) group
    to a multiple of 128 edges, common across cores.

    Returns:
      segs:   per dst-tile, list of (half, ntiles)
      idx16:  per-core [128, NF*8] int16 gather indices (16-partition
              wrapped layout for dma_gather, replicated 8x vertically)
      val:    per-core [P, NF] f32
      dl:     per-core [P, NF] f32 (dst_local in 0..127)
    """
    dst_sh = n_dst // ncores
    nt = dst_sh // P
    nhalf = 2 if split else 1
    half_rows = n_gather // nhalf

    core_of = dst // dst_sh
    tile_of = (dst % dst_sh) // P
    dl_of = (dst % P).astype(np.float32)
    half_of = (gidx // half_rows) if split else np.zeros(len(dst), np.int64)
    lidx = (gidx - half_of * half_rows).astype(np.int16)

    grp = (core_of * nt + tile_of) * nhalf + half_of
    order = np.lexsort((gidx, grp))
    so_lidx = lidx[order]
    so_val = vals[order].astype(np.float32)
    so_dl = dl_of[order]

    ngrp = ncores * nt * nhalf
    counts = np.bincount(grp, minlength=ngrp).reshape(ncores, nt, nhalf)
    # per (tile, half) tile count, common across cores; >=1 tile per dst tile
    ntile_th = np.ceil(counts.max(axis=0) / P).astype(np.int64)  # [nt, nhalf]
    for t in range(nt):
        if ntile_th[t].sum() == 0:
            ntile_th[t, 0] = 1
    nf = int(ntile_th.sum())
    off_flat = np.concatenate([[0], np.cumsum(ntile_th.reshape(-1))])

    i_arr = np.zeros((ncores, nf * P), dtype=np.int16)
    v_arr = np.zeros((ncores, nf * P), dtype=np.float32)
    d_arr = np.zeros((ncores, nf * P), dtype=np.float32)

    grp_start = np.concatenate([[0], np.cumsum(counts.reshape(-1))])
    for c in range(ncores):
        for t in range(nt):
            for h in range(nhalf):
                gi = (c * nt + t) * nhalf + h
                s, e = grp_start[gi], grp_start[gi + 1]
                n = e - s
                o = off_flat[t * nhalf + h] * P
                i_arr[c, o : o + n] = so_lidx[s:e]
                v_arr[c, o : o + n] = so_val[s:e]
                d_arr[c, o : o + n] = so_dl[s:e]

    segs = [
        [(h, int(ntile_th[t, h])) for h in range(nhalf) if ntile_th[t, h] > 0]
        for t in range(nt)
    ]
    # dma_gather index layout: linear idx i at [i % 16, i // 16], x8 vertical
    idx16 = []
    for c in range(ncores):
        a = i_arr[c].reshape(nf * 8, 16).T  # [16, nf*8]
        idx16.append(np.ascontiguousarray(np.tile(a, (8, 1))))
    v_dev = [np.ascontiguousarray(v_arr[c].reshape(nf, P).T) for c in range(ncores)]
    d_dev = [np.ascontiguousarray(d_arr[c].reshape(nf, P).T) for c in range(ncores)]
    return segs, idx16, v_dev, d_dev


# ----------------------------------------------------------------- bass build


def _install_drain_patch():
    """walrus in this env allows only ONE sem-wait per instruction; split
    extra waits onto same-engine carrier instructions."""
    import concourse.mybir as mybir
    import concourse.tile as _tile
    from concourse.vector_clock import ScopedClock

    if getattr(_tile.TileContext, "_drain_split_patched", False):
        return

    def _split_drain_and_barrier(self, tick_clock, wait_clock):
        nc = self.nc
        drain_inst = nc.sync.drain()
        wait_clock.add_sem_waits(
            drain_inst.ins, ScopedClock({None: tick_clock.global_clock})
        )
        si = drain_inst.ins.sync_info
        waits = list(si.on_wait) if si and si.on_wait else []
        if len(waits) > 1:
            si.on_wait = waits[:1]
            drain_inst.ins.sync_info = si
            for i in range(1, len(waits)):
                extra = nc.sync.drain()
                esi = extra.ins.sync_info
                upd = list(esi.on_update) if esi and esi.on_update else []
                extra.ins.sync_info = mybir.SyncInfo(
                    on_wait=[waits[i]], on_update=upd
                )
        nc.all_engine_barrier()
        assert self.sems is not None
        popped = nc._tile_sem_poison_stack.pop()
        assert popped is self._sem_poison
        nc.clear_and_free_semaphores(list(self.sems.allocated().values()))
        nc.all_engine_barrier()

    _tile.TileContext._drain_and_barrier = _split_drain_and_barrier

    _orig_add = _tile.TileContext._add_instruction

    def _add_instruction_split(self, inst):
        si = inst.sync_info
        waits = list(si.on_wait) if si and si.on_wait else []
        if len(waits) > 1 and inst.engine != mybir.EngineType.Unassigned:
            for w in waits[:-1]:
                nop = mybir.InstNoOp(
                    name=self.nc.get_next_instruction_name(), ins=[], outs=[]
                )
                nop.engine = inst.engine
                nop.sync_info = mybir.SyncInfo(on_wait=[w], on_update=[])
                _orig_add(self, nop)
            si.on_wait = waits[-1:]
            inst.sync_info = si
        _orig_add(self, inst)

    _tile.TileContext._add_instruction = _add_instruction_split
    _tile.TileContext._drain_split_patched = True


def _build_program(n_tgt, n_src, fsegs, bsegs, taps=False):
    """Build the SPMD bass program (identical on all 8 cores)."""
    from contextlib import ExitStack

    import concourse.bass as bass
    import concourse.mybir as mybir
    import concourse.tile as tile
    from concourse import bacc
    from concourse.masks import make_identity

    _install_drain_patch()

    dt = mybir.dt
    f32 = dt.float32
    i16 = dt.int16
    tb = getattr(dt, _TABLE_DT)
    D = D_FIXED
    DC = D // P
    tgt_sh = n_tgt // NCORES
    src_sh = n_src // NCORES
    NT = tgt_sh // P
    NF = sum(n for seg in fsegs for _, n in seg)
    NB = sum(n for seg in bsegs for _, n in seg)
    AluOp = mybir.AluOpType
    Act = mybir.ActivationFunctionType
    rg = [list(range(NCORES))]

    nc = bacc.Bacc("TRN2", target_bir_lowering=False, debug=False, num_devices=NCORES)

    dram_t = nc.dram_tensor
    HsrcT = dram_t("HsrcT", [D, n_src], f32, kind="ExternalInput").ap()
    W0 = dram_t("W0", [D, D], f32, kind="ExternalInput").ap()
    Wb = dram_t("Wb", [D, D], f32, kind="ExternalInput").ap()
    W1 = dram_t("W1", [D, D], f32, kind="ExternalInput").ap()
    b0_h = dram_t("b0", [1, D], f32, kind="ExternalInput")
    bb_h = dram_t("bb", [1, D], f32, kind="ExternalInput")
    b1_h = dram_t("b1", [1, D], f32, kind="ExternalInput")
    g1T = dram_t("g1T", [P, DC], f32, kind="ExternalInput").ap()
    be1T = dram_t("be1T", [P, DC], f32, kind="ExternalInput").ap()
    g2T = dram_t("g2T", [P, DC], f32, kind="ExternalInput").ap()
    be2T = dram_t("be2T", [P, DC], f32, kind="ExternalInput").ap()
    iota_d = dram_t("iota", [P, P], f32, kind="ExternalInput").ap()
    emb = dram_t("emb", [tgt_sh, D], f32, kind="ExternalInput").ap()
    fe_i16 = dram_t("fe_i16", [P, NF * 8], i16, kind="ExternalInput").ap()
    fe_val = dram_t("fe_val", [P, NF], f32, kind="ExternalInput").ap()
    fe_dl = dram_t("fe_dl", [P, NF], f32, kind="ExternalInput").ap()
    be_i16 = dram_t("be_i16", [P, NB * 8], i16, kind="ExternalInput").ap()
    be_val = dram_t("be_val", [P, NB], f32, kind="ExternalInput").ap()
    be_dl = dram_t("be_dl", [P, NB], f32, kind="ExternalInput").ap()
    outT = dram_t("outT", [D, tgt_sh], f32, kind="ExternalOutput").ap()
    if taps:
        dbg_wh0 = dram_t("dbg_wh0", [n_src, D], tb, kind="ExternalOutput").ap()
        dbg_x1T = dram_t("dbg_x1T", [D, tgt_sh], f32, kind="ExternalOutput").ap()
        dbg_whb = dram_t("dbg_whb", [n_tgt, D], tb, kind="ExternalOutput").ap()
        dbg_wh1 = dram_t("dbg_wh1", [n_src, D], tb, kind="ExternalOutput").ap()
        dbg_st1 = dram_t("dbg_st1", [P, 4], f32, kind="ExternalOutput").ap()
        dbg_x1pre = dram_t("dbg_x1pre", [D, tgt_sh], f32, kind="ExternalOutput").ap()
        dbg_deg = dram_t("dbg_deg", [P, NT], f32, kind="ExternalOutput").ap()

    with tile.TileContext(nc) as tc, ExitStack() as ctx:
        dram = ctx.enter_context(tc.tile_pool(name="dram", bufs=1, space="DRAM"))
        half_rows = n_src // 2
        WH0_t = [dram.tile([half_rows, D], tb, name=f"WH0h{h}") for h in range(2)]
        WHb_loc = dram.tile([tgt_sh, D], tb)
        WHb_full = dram.tile([n_tgt, D], tb, addr_space="Shared")
        WH1_loc = dram.tile([src_sh, D], tb)
        WH1_full = dram.tile([n_src, D], tb, addr_space="Shared")
        WH1_t = [
            WH1_full[h * half_rows : (h + 1) * half_rows, :] for h in range(2)
        ]
        st1_in = dram.tile([P, 2 * DC], f32)
        st1_out = dram.tile([P, 2 * DC], f32, addr_space="Shared")
        st2_in = dram.tile([P, 2 * DC], f32)
        st2_out = dram.tile([P, 2 * DC], f32, addr_space="Shared")

        consts = ctx.enter_context(tc.tile_pool(name="consts", bufs=1))
        w0t = consts.tile([P, DC, D], f32)
        wbt = consts.tile([P, DC, D], f32)
        w1t = consts.tile([P, DC, D], f32)
        for c in range(DC):
            nc.sync.dma_start(out=w0t[:, c, :], in_=W0[c * P : (c + 1) * P, :])
            nc.sync.dma_start(out=wbt[:, c, :], in_=Wb[c * P : (c + 1) * P, :])
            nc.sync.dma_start(out=w1t[:, c, :], in_=W1[c * P : (c + 1) * P, :])
        w0b = consts.tile([P, DC, D], tb)
        wbb = consts.tile([P, DC, D], tb)
        for c in range(DC):
            nc.vector.tensor_copy(out=w0b[:, c, :], in_=w0t[:, c, :])
            nc.vector.tensor_copy(out=wbb[:, c, :], in_=wbt[:, c, :])
        b0bc = consts.tile([P, D], f32)
        bbbc = consts.tile([P, D], f32)
        b1bc = consts.tile([P, D], f32)
        for h_, t_ in ((b0_h, b0bc), (bb_h, bbbc), (b1_h, b1bc)):
            nc.gpsimd.dma_start(
                out=t_[:], in_=bass.AP(tensor=h_, offset=0, ap=[[0, P], [1, D]])
            )
        g1f = consts.tile([P, DC], f32)
        be1f = consts.tile([P, DC], f32)
        g2f = consts.tile([P, DC], f32)
        be2f = consts.tile([P, DC], f32)
        nc.sync.dma_start(out=g1f[:], in_=g1T[:])
        nc.sync.dma_start(out=be1f[:], in_=be1T[:])
        nc.sync.dma_start(out=g2f[:], in_=g2T[:])
        nc.sync.dma_start(out=be2f[:], in_=be2T[:])
        iota_t = consts.tile([P, P], f32)
        nc.sync.dma_start(out=iota_t[:], in_=iota_d[:])
        epst = consts.tile([P, 1], f32)
        nc.vector.memset(epst[:], EPS)
        ident = consts.tile([P, P], f32)
        make_identity(nc, ident[:])
        onesb = consts.tile([P, 1], tb)
        nc.vector.memset(onesb[:], 1.0)

        # resident state
        xT = consts.tile([P, DC, NT, P], tb)  # H_tgt1 (feature-major)
        x2T = consts.tile([P, DC, NT, P], f32)  # layer-1 pre-BN x
        degc = consts.tile([P, NT], f32)  # reciprocal clamped target degree

        sv_pool = ctx.enter_context(tc.tile_pool(name="sv", bufs=8))
        g_pool = ctx.enter_context(tc.tile_pool(name="gp", bufs=3))
        ps_acc = ctx.enter_context(tc.tile_pool(name="psacc", bufs=3, space="PSUM"))
        ps_tr = ctx.enter_context(tc.tile_pool(name="pstr", bufs=2, space="PSUM"))
        hrm = ctx.enter_context(tc.tile_pool(name="hrm", bufs=4))
        whs_pool = ctx.enter_context(tc.tile_pool(name="whs", bufs=4))
        lhs_pool = ctx.enter_context(tc.tile_pool(name="lhs", bufs=2))
        misc = ctx.enter_context(tc.tile_pool(name="misc", bufs=6))

        # ---------------- phase A: full WH0 = H_src @ W0 (every core) ----
        SUP = 512 if n_src % 512 == 0 else P
        for st in range(n_src // SUP):
            haf = lhs_pool.tile([P, SUP], f32, tag="haf")
            hbf = lhs_pool.tile([P, SUP], f32, tag="hbf")
            nc.sync.dma_start(out=haf[:], in_=HsrcT[0:P, st * SUP : (st + 1) * SUP])
            nc.sync.dma_start(
                out=hbf[:], in_=HsrcT[P : 2 * P, st * SUP : (st + 1) * SUP]
            )
            ha = lhs_pool.tile([P, SUP], tb, tag="ha")
            hb = lhs_pool.tile([P, SUP], tb, tag="hb")
            nc.vector.tensor_copy(out=ha[:], in_=haf[:])
            nc.vector.tensor_copy(out=hb[:], in_=hbf[:])
            for r in range(SUP // P):
                ps = ps_acc.tile([P, D], f32, tag="acc")
                nc.tensor.matmul(
                    out=ps[:],
                    lhsT=ha[:, r * P : (r + 1) * P],
                    rhs=w0b[:, 0, :],
                    start=True,
                    stop=False,
                )
                nc.tensor.matmul(
                    out=ps[:],
                    lhsT=hb[:, r * P : (r + 1) * P],
                    rhs=w0b[:, 1, :],
                    start=False,
                    stop=True,
                )
                whs = whs_pool.tile([P, D], tb, tag="whs")
                nc.scalar.copy(out=whs[:], in_=ps[:])
                row0 = (st * (SUP // P) + r) * P
                h = row0 // half_rows
                lr = row0 - h * half_rows
                nc.sync.dma_start(out=WH0_t[h][lr : lr + P, :], in_=whs[:])

        # ---------------- gather pass helper ----------------------------
        def gather_pass(segs, idx_tile, val_tile, dl_tile, tables, want_deg, post):
            """segs: per dst-tile list of (table_idx, ntiles)."""
            j = 0  # global edge-tile index
            for t in range(len(segs)):
                ntile_tot = sum(n for _, n in segs[t])
                ps = ps_acc.tile([P, D], f32, tag="acc", name=f"ps{t}")
                psd = None
                if want_deg:
                    psd = ps_acc.tile([P, 1], f32, tag="deg", bufs=2, name=f"psd{t}")
                k = 0  # tile index within dst-tile
                for tab_i, nseg in segs[t]:
                    table = tables[tab_i]
                    done = 0
                    while done < nseg:
                        bs = min(GBT, nseg - done)
                        gt = g_pool.tile([P, GBT, D], tb, tag="gt", name="gt")
                        nc.gpsimd.dma_gather(
                            out_ap=gt[:, 0:bs, :],
                            in_ap=table,
                            idxs_ap=idx_tile[:, j * 8 : (j + bs) * 8],
                            num_idxs=bs * P,
                            num_idxs_reg=bs * P,
                            elem_size=D,
                        )
                        for i in range(bs):
                            jj = j + i
                            sv = sv_pool.tile([P, P], tb, name="sv")
                            nc.vector.scalar_tensor_tensor(
                                out=sv[:],
                                in0=iota_t[:],
                                scalar=dl_tile[:, jj : jj + 1],
                                in1=val_tile[:, jj : jj + 1].to_broadcast([P, P]),
                                op0=AluOp.is_equal,
                                op1=AluOp.mult,
                            )
                            nc.tensor.matmul(
                                out=ps[:],
                                lhsT=sv[:],
                                rhs=gt[:, i, :],
                                start=(k + i == 0),
                                stop=(k + i == ntile_tot - 1),
                            )
                            if psd is not None:
                                nc.tensor.matmul(
                                    out=psd[:],
                                    lhsT=sv[:],
                                    rhs=onesb[:],
                                    start=(k + i == 0),
                                    stop=(k + i == ntile_tot - 1),
                                )
                        j += bs
                        done += bs
                        k += bs
                post(t, ps, psd)
            return j

        # ---------------- pass B: layer-0 forward ------------------------
        def post_fwd0(t, ps, psd):
            dtmp = misc.tile([P, 1], f32, tag="dtgt")
            nc.vector.tensor_scalar_max(dtmp[:], psd[:], 1.0)
            nc.vector.reciprocal(degc[:, t : t + 1], dtmp[:])
            h1 = hrm.tile([P, D], f32, tag="h1")
            nc.vector.scalar_tensor_tensor(
                out=h1[:],
                in0=ps[:],
                scalar=degc[:, t : t + 1],
                in1=b0bc[:],
                op0=AluOp.mult,
                op1=AluOp.add,
            )
            h2 = hrm.tile([P, D], f32, tag="h2")
            nc.scalar.activation(out=h2[:], in_=h1[:], func=Act.Relu)
            et = misc.tile([P, D], f32, tag="emb")
            nc.sync.dma_start(out=et[:], in_=emb[t * P : (t + 1) * P, :])
            xr = hrm.tile([P, D], f32, tag="xr")
            nc.vector.tensor_add(xr[:], h2[:], et[:])
            for c in range(DC):
                pt = ps_tr.tile([P, P], f32, tag="tr")
                nc.tensor.transpose(
                    out=pt[:], in_=xr[:, c * P : (c + 1) * P], identity=ident[:]
                )
                nc.vector.tensor_copy(out=xT[:, c, t, :], in_=pt[:])

        with tc.tile_pool(name="edgesB", bufs=1) as ep:
            fidx = ep.tile([P, NF * 8], i16, name="fidxB")
            fval = ep.tile([P, NF], f32, name="fvalB")
            fdl = ep.tile([P, NF], f32, name="fdlB")
            nc.sync.dma_start(out=fidx[:], in_=fe_i16[:])
            nc.sync.dma_start(out=fval[:], in_=fe_val[:])
            nc.sync.dma_start(out=fdl[:], in_=fe_dl[:])
            gather_pass(
                fsegs, fidx, fval, fdl, [t_[:] for t_ in WH0_t], True, post_fwd0
            )

        # ---------------- BN helpers -------------------------------------
        def bn_stats_phase(xbuf, count, st_in_sb_name):
            st_sb = misc.tile([P, 2 * DC], f32, name=st_in_sb_name, tag="stats")
            grp = min(512, count)
            ngrp = count // grp
            for c in range(DC):
                bnst = misc.tile([P, ngrp, 6], f32, tag="bnst")
                flat = xbuf[:, c, :, :].rearrange("p a b -> p (a b)")
                for g in range(ngrp):
                    nc.vector.bn_stats(
                        out=bnst[:, g, :], in_=flat[:, g * grp : (g + 1) * grp]
                    )
                mv = misc.tile([P, 2], f32, tag="mv")
                nc.vector.bn_aggr(out=mv[:], in_=bnst[:].rearrange("p a b -> p (a b)"))
                nc.vector.tensor_scalar_mul(
                    st_sb[:, 2 * c : 2 * c + 1], mv[:, 0:1], float(count)
                )
                musq = misc.tile([P, 1], f32, tag="musq")
                nc.vector.tensor_mul(musq[:], mv[:, 0:1], mv[:, 0:1])
                nc.vector.tensor_add(musq[:], musq[:], mv[:, 1:2])
                nc.vector.tensor_scalar_mul(
                    st_sb[:, 2 * c + 1 : 2 * c + 2], musq[:], float(count)
                )
            return st_sb

        def bn_coeffs(st_full_sb, gamma_f, beta_f, total, a_name, b_name):
            A = misc.tile([P, DC], f32, name=a_name, tag="bnA")
            B = misc.tile([P, DC], f32, name=b_name, tag="bnB")
            for c in range(DC):
                mu = misc.tile([P, 1], f32, tag="mu")
                nc.vector.tensor_scalar_mul(
                    mu[:], st_full_sb[:, 2 * c : 2 * c + 1], 1.0 / total
                )
                q = misc.tile([P, 1], f32, tag="q")
                nc.vector.tensor_scalar_mul(
                    q[:], st_full_sb[:, 2 * c + 1 : 2 * c + 2], 1.0 / total
                )
                musq = misc.tile([P, 1], f32, tag="musq2")
                nc.vector.tensor_mul(musq[:], mu[:], mu[:])
                var = misc.tile([P, 1], f32, tag="var")
                nc.vector.tensor_tensor(
                    out=var[:], in0=q[:], in1=musq[:], op=AluOp.subtract
                )
                sd = misc.tile([P, 1], f32, tag="sd")
                nc.scalar.activation(out=sd[:], in_=var[:], func=Act.Sqrt, bias=epst[:])
                rstd = misc.tile([P, 1], f32, tag="rstd")
                nc.vector.reciprocal(rstd[:], sd[:])
                nc.vector.tensor_mul(A[:, c : c + 1], gamma_f[:, c : c + 1], rstd[:])
                mA = misc.tile([P, 1], f32, tag="mA")
                nc.vector.tensor_mul(mA[:], mu[:], A[:, c : c + 1])
                nc.vector.tensor_tensor(
                    out=B[:, c : c + 1],
                    in0=beta_f[:, c : c + 1],
                    in1=mA[:],
                    op=AluOp.subtract,
                )
            return A, B

        if taps:
            for h in range(2):
                nc.sync.dma_start(
                    out=dbg_wh0[h * half_rows : (h + 1) * half_rows, :],
                    in_=WH0_t[h][:],
                )
            nc.sync.dma_start(out=dbg_deg[:], in_=degc[:])
            for c in range(DC):
                nc.sync.dma_start(
                    out=dbg_x1pre[c * P : (c + 1) * P, :],
                    in_=xT[:, c, :, :].rearrange("p a b -> p (a b)"),
                )

        # ---------------- BN-1 + WHb + AllGather -------------------------
        st1_sb = bn_stats_phase(xT, tgt_sh, "st1_sb")
        nc.sync.dma_start(out=st1_in[:], in_=st1_sb[:])
        nc.gpsimd.collective_compute(
            "AllReduce",
            AluOp.add,
            replica_groups=rg,
            ins=[st1_in[:].opt()],
            outs=[st1_out[:].opt()],
        )
        st1g = misc.tile([P, 2 * DC], f32, tag="stg")
        nc.sync.dma_start(out=st1g[:], in_=st1_out[:])
        A1, B1 = bn_coeffs(st1g, g1f, be1f, n_tgt, "A1", "B1")

        for t in range(NT):
            for c in range(DC):
                nc.vector.scalar_tensor_tensor(
                    out=xT[:, c, t, :],
                    in0=xT[:, c, t, :],
                    scalar=A1[:, c : c + 1],
                    in1=B1[:, c : c + 1].to_broadcast([P, P]),
                    op0=AluOp.mult,
                    op1=AluOp.add,
                )
            ps = ps_acc.tile([P, D], f32, tag="acc", name=f"pswb{t}")
            nc.tensor.matmul(
                out=ps[:], lhsT=xT[:, 0, t, :], rhs=wbb[:, 0, :], start=True, stop=False
            )
            nc.tensor.matmul(
                out=ps[:], lhsT=xT[:, 1, t, :], rhs=wbb[:, 1, :], start=False, stop=True
            )
            whs = whs_pool.tile([P, D], tb, tag="whs")
            nc.scalar.copy(out=whs[:], in_=ps[:])
            nc.sync.dma_start(out=WHb_loc[t * P : (t + 1) * P, :], in_=whs[:])

        nc.gpsimd.collective_compute(
            "AllGather",
            AluOp.bypass,
            replica_groups=rg,
            ins=[WHb_loc[:].opt()],
            outs=[WHb_full[:].opt()],
        )
        if taps:
            nc.sync.dma_start(out=dbg_st1[:], in_=st1_out[:])
            for c in range(DC):
                nc.sync.dma_start(
                    out=dbg_x1T[c * P : (c + 1) * P, :],
                    in_=xT[:, c, :, :].rearrange("p a b -> p (a b)"),
                )
            nc.sync.dma_start(out=dbg_whb[:], in_=WHb_full[:])

        # ---------------- pass E: layer-0 backward (+ fused WH1) ---------
        def post_bwd(t, ps, psd):
            dtmp = misc.tile([P, 1], f32, tag="dsrc")
            nc.vector.tensor_scalar_max(dtmp[:], psd[:], 1.0)
            rtmp = misc.tile([P, 1], f32, tag="rsrc")
            nc.vector.reciprocal(rtmp[:], dtmp[:])
            h1 = hrm.tile([P, D], f32, tag="h1")
            nc.vector.scalar_tensor_tensor(
                out=h1[:],
                in0=ps[:],
                scalar=rtmp[:],
                in1=bbbc[:],
                op0=AluOp.mult,
                op1=AluOp.add,
            )
            h2 = hrm.tile([P, D], f32, tag="h2")
            nc.scalar.activation(out=h2[:], in_=h1[:], func=Act.Relu)
            hsb = misc.tile([P, DC, P], f32, tag="hsb")
            for c in range(DC):
                pt = ps_tr.tile([P, P], f32, tag="tr")
                nc.tensor.transpose(
                    out=pt[:], in_=h2[:, c * P : (c + 1) * P], identity=ident[:]
                )
                nc.vector.tensor_copy(out=hsb[:, c, :], in_=pt[:])
            ps2 = ps_acc.tile([P, D], f32, tag="acc", name=f"psw1{t}")
            nc.tensor.matmul(
                out=ps2[:], lhsT=hsb[:, 0, :], rhs=w1t[:, 0, :], start=True, stop=False
            )
            nc.tensor.matmul(
                out=ps2[:], lhsT=hsb[:, 1, :], rhs=w1t[:, 1, :], start=False, stop=True
            )
            whs = whs_pool.tile([P, D], tb, tag="whs")
            nc.scalar.copy(out=whs[:], in_=ps2[:])
            nc.sync.dma_start(out=WH1_loc[t * P : (t + 1) * P, :], in_=whs[:])

        with tc.tile_pool(name="edgesE", bufs=1) as ep:
            bidx = ep.tile([P, NB * 8], i16, name="bidxE")
            bval = ep.tile([P, NB], f32, name="bvalE")
            bdl = ep.tile([P, NB], f32, name="bdlE")
            nc.sync.dma_start(out=bidx[:], in_=be_i16[:])
            nc.sync.dma_start(out=bval[:], in_=be_val[:])
            nc.sync.dma_start(out=bdl[:], in_=be_dl[:])
            gather_pass(bsegs, bidx, bval, bdl, [WHb_full[:]], True, post_bwd)

        nc.gpsimd.collective_compute(
            "AllGather",
            AluOp.bypass,
            replica_groups=rg,
            ins=[WH1_loc[:].opt()],
            outs=[WH1_full[:].opt()],
        )
        if taps:
            nc.sync.dma_start(out=dbg_wh1[:], in_=WH1_full[:])

        # ---------------- pass G: layer-1 forward ------------------------
        def post_fwd1(t, ps, psd):
            h1 = hrm.tile([P, D], f32, tag="h1")
            nc.vector.scalar_tensor_tensor(
                out=h1[:],
                in0=ps[:],
                scalar=degc[:, t : t + 1],
                in1=b1bc[:],
                op0=AluOp.mult,
                op1=AluOp.add,
            )
            h2 = hrm.tile([P, D], f32, tag="h2")
            nc.scalar.activation(out=h2[:], in_=h1[:], func=Act.Relu)
            for c in range(DC):
                pt = ps_tr.tile([P, P], f32, tag="tr")
                nc.tensor.transpose(
                    out=pt[:], in_=h2[:, c * P : (c + 1) * P], identity=ident[:]
                )
                nc.vector.tensor_add(x2T[:, c, t, :], pt[:], xT[:, c, t, :])

        with tc.tile_pool(name="edgesG", bufs=1) as ep:
            fidx2 = ep.tile([P, NF * 8], i16, name="fidxG")
            fval2 = ep.tile([P, NF], f32, name="fvalG")
            fdl2 = ep.tile([P, NF], f32, name="fdlG")
            nc.sync.dma_start(out=fidx2[:], in_=fe_i16[:])
            nc.sync.dma_start(out=fval2[:], in_=fe_val[:])
            nc.sync.dma_start(out=fdl2[:], in_=fe_dl[:])
            gather_pass(
                fsegs, fidx2, fval2, fdl2, [t_[:] for t_ in WH1_t], False, post_fwd1
            )

        # ---------------- BN-2 + output ----------------------------------
        st2_sb = bn_stats_phase(x2T, tgt_sh, "st2_sb")
        nc.sync.dma_start(out=st2_in[:], in_=st2_sb[:])
        nc.gpsimd.collective_compute(
            "AllReduce",
            AluOp.add,
            replica_groups=rg,
            ins=[st2_in[:].opt()],
            outs=[st2_out[:].opt()],
        )
        st2g = misc.tile([P, 2 * DC], f32, tag="stg")
        nc.sync.dma_start(out=st2g[:], in_=st2_out[:])
        A2, B2 = bn_coeffs(st2g, g2f, be2f, n_tgt, "A2", "B2")

        for t in range(NT):
            for c in range(DC):
                nc.vector.scalar_tensor_tensor(
                    out=x2T[:, c, t, :],
                    in0=x2T[:, c, t, :],
                    scalar=A2[:, c : c + 1],
                    in1=B2[:, c : c + 1].to_broadcast([P, P]),
                    op0=AluOp.mult,
                    op1=AluOp.add,
                )
        for c in range(DC):
            nc.sync.dma_start(
                out=outT[c * P : (c + 1) * P, :],
                in_=x2T[:, c, :, :].rearrange("p a b -> p (a b)"),
            )

    nc.compile()
    return nc


# ----------------------------------------------------------------- entry


def _run(inputs, trace=False, tmpdir=None, taps=False):
    from concourse.bass_utils import run_bass_kernel_spmd

    H_src = np.asarray(inputs["H_src"], dtype=np.float32)
    target_emb = np.asarray(inputs["target_emb"], dtype=np.float32)
    W_fwd = np.asarray(inputs["W_fwd"], dtype=np.float32)
    b_fwd = np.asarray(inputs["b_fwd"], dtype=np.float32)
    W_bwd = np.asarray(inputs["W_bwd"], dtype=np.float32)
    b_bwd = np.asarray(inputs["b_bwd"], dtype=np.float32)
    gamma = np.asarray(inputs["gamma"], dtype=np.float32)
    beta = np.asarray(inputs["beta"], dtype=np.float32)
    vals = np.asarray(inputs["vals"], dtype=np.float32)
    rows = np.asarray(inputs["rows"])
    cols = np.asarray(inputs["cols"])

    n_src, D = H_src.shape
    n_tgt = target_emb.shape[0]
    assert D == D_FIXED
    tgt_sh = n_tgt // NCORES
    DC = D // P

    fsegs, f_i, f_v, f_d = _edge_plan(
        rows, cols, vals, n_tgt, n_src, NCORES, split=True
    )
    bsegs, b_i, b_v, b_d = _edge_plan(
        cols, rows, vals, n_src, n_tgt, NCORES, split=False
    )

    nc = _build_program(n_tgt, n_src, fsegs, bsegs, taps=taps)

    HsrcT = np.ascontiguousarray(H_src.T)
    iota = np.ascontiguousarray(np.tile(np.arange(P, dtype=np.float32), (P, 1)))

    def fmaj(v):  # [D] -> [P, DC] feature-major
        return np.ascontiguousarray(v.reshape(DC, P).T)

    in_maps = []
    for c in range(NCORES):
        in_maps.append(
            {
                "HsrcT": HsrcT,
                "W0": W_fwd[0],
                "Wb": W_bwd[0],
                "W1": W_fwd[1],
                "b0": b_fwd[0].reshape(1, D),
                "bb": b_bwd[0].reshape(1, D),
                "b1": b_fwd[1].reshape(1, D),
                "g1T": fmaj(gamma[0]),
                "be1T": fmaj(beta[0]),
                "g2T": fmaj(gamma[1]),
                "be2T": fmaj(beta[1]),
                "iota": iota,
                "emb": np.ascontiguousarray(
                    target_emb[c * tgt_sh : (c + 1) * tgt_sh]
                ),
                "fe_i16": f_i[c],
                "fe_val": f_v[c],
                "fe_dl": f_d[c],
                "be_i16": b_i[c],
                "be_val": b_v[c],
                "be_dl": b_d[c],
            }
        )

    res = run_bass_kernel_spmd(
        nc, in_maps, list(range(NCORES)), trace=trace, tmpdir=tmpdir
    )
    out = np.concatenate(
        [np.asarray(res.results[c]["outT"]).astype(np.float32).T for c in range(NCORES)],
        axis=0,
    )
    return np.ascontiguousarray(out), res


def kernel(**inputs) -> np.ndarray:
    out, _ = _run(inputs)
    return out


# revision 25
# speedup vs baseline: 1.2493x; 1.0141x over previous
"""Bipartite GCN stack (2 layers) on 8 Trainium2 NeuronCores.

Strategy (graph/data parallel, destination-sharded):
  - Layer-0 forward: every core computes the full WH0 = H_src @ W_fwd0
    (redundantly; cheaper than all-gathering the 64MB table), then
    processes the ~1/8 of edges whose destination (row) falls in its
    4096-target shard.  segment_sum is done by gathering WH0 rows in
    1024-row batches with the Q7 dma_gather instruction and reducing
    each 128-edge tile into PSUM with a selection-matrix matmul
    (S[e,d] = val[e] if dst_local[e]==d).  A second tiny matmul against
    a ones column accumulates the degrees.
  - dma_gather needs int16 indices, so 65536-row tables are split into
    lo/hi halves of 32768 rows and each destination tile's edges are
    grouped by half (host-side index preprocessing only).
  - BatchNorm: per-core partial sums via bn_stats in feature-major
    layout, 2KB AllReduce, scale/shift applied in feature-major form
    (which is exactly the lhsT layout the next dense matmul needs).
  - Backward (layer 0 only; the layer-1 backward output is dead):
    AllGather of WHb, edges sharded by source, same reduction.
  - Layer-1 forward: AllGather of WH1, reuse of the layer-0 target
    degrees, residual in feature-major form, BN, output.

Gathered tables are stored in _TABLE_DT (bfloat16 by default: 4x PE
matmul rate, half the gather/collective bytes); everything else is
fp32.  Host-side work is limited to sharding/permutation of the edge
index arrays and layout transforms of inputs/outputs; all
floating-point math runs on the NeuronCores.
"""

import numpy as np

P = 128
D_FIXED = 256
EPS = 1e-5
NCORES = 8
GBT = 8  # gather batch: edge tiles per dma_gather (1024 rows)

# dtype of the gathered feature tables (WH0/WHb/WH1), the gather buffers
# and the selection matrices: "float32" (exact) or "bfloat16"
_TABLE_DT = "bfloat16"


# ----------------------------------------------------------------- host prep


def _edge_plan(dst, gidx, vals, n_dst, n_gather, ncores, split):
    """Partition edges by destination shard, group by 128-row dst tile and
    (optionally) by gather-table half; pad each (core, tile[, half]) group
    to a multiple of 128 edges, common across cores.

    Returns:
      segs:   per dst-tile, list of (half, ntiles)
      idx16:  per-core [128, NF*8] int16 gather indices (16-partition
              wrapped layout for dma_gather, replicated 8x vertically)
      val:    per-core [P, NF] f32
      dl:     per-core [P, NF] f32 (dst_local in 0..127)
    """
    dst_sh = n_dst // ncores
    nt = dst_sh // P
    nhalf = 2 if split else 1
    half_rows = n_gather // nhalf

    core_of = dst // dst_sh
    tile_of = (dst % dst_sh) // P
    dl_of = (dst % P).astype(np.float32)
    half_of = (gidx // half_rows) if split else np.zeros(len(dst), np.int64)
    lidx = (gidx - half_of * half_rows).astype(np.int16)

    grp = (core_of * nt + tile_of) * nhalf + half_of
    order = np.lexsort((gidx, grp))
    so_lidx = lidx[order]
    so_val = vals[order].astype(np.float32)
    so_dl = dl_of[order]

    ngrp = ncores * nt * nhalf
    counts = np.bincount(grp, minlength=ngrp).reshape(ncores, nt, nhalf)
    # per (tile, half) tile count, common across cores; >=1 tile per dst tile
    ntile_th = np.ceil(counts.max(axis=0) / P).astype(np.int64)  # [nt, nhalf]
    for t in range(nt):
        if ntile_th[t].sum() == 0:
            ntile_th[t, 0] = 1
    nf = int(ntile_th.sum())
    off_flat = np.concatenate([[0], np.cumsum(ntile_th.reshape(-1))])

    i_arr = np.zeros((ncores, nf * P), dtype=np.int16)
    v_arr = np.zeros((ncores, nf * P), dtype=np.float32)
    d_arr = np.zeros((ncores, nf * P), dtype=np.float32)

    grp_start = np.concatenate([[0], np.cumsum(counts.reshape(-1))])
    for c in range(ncores):
        for t in range(nt):
            for h in range(nhalf):
                gi = (c * nt + t) * nhalf + h
                s, e = grp_start[gi], grp_start[gi + 1]
                n = e - s
                o = off_flat[t * nhalf + h] * P
                i_arr[c, o : o + n] = so_lidx[s:e]
                v_arr[c, o : o + n] = so_val[s:e]
                d_arr[c, o : o + n] = so_dl[s:e]

    segs = [
        [(h, int(ntile_th[t, h])) for h in range(nhalf) if ntile_th[t, h] > 0]
        for t in range(nt)
    ]
    # dma_gather index layout: linear idx i at [i % 16, i // 16], x8 vertical
    idx16 = []
    for c in range(ncores):
        a = i_arr[c].reshape(nf * 8, 16).T  # [16, nf*8]
        idx16.append(np.ascontiguousarray(np.tile(a, (8, 1))))
    v_dev = [np.ascontiguousarray(v_arr[c].reshape(nf, P).T) for c in range(ncores)]
    d_dev = [np.ascontiguousarray(d_arr[c].reshape(nf, P).T) for c in range(ncores)]
    return segs, idx16, v_dev, d_dev


# ----------------------------------------------------------------- bass build


def _install_drain_patch():
    """walrus in this env allows only ONE sem-wait per instruction; split
    extra waits onto same-engine carrier instructions."""
    import concourse.mybir as mybir
    import concourse.tile as _tile
    from concourse.vector_clock import ScopedClock

    if getattr(_tile.TileContext, "_drain_split_patched", False):
        return

    def _split_drain_and_barrier(self, tick_clock, wait_clock):
        nc = self.nc
        drain_inst = nc.sync.drain()
        wait_clock.add_sem_waits(
            drain_inst.ins, ScopedClock({None: tick_clock.global_clock})
        )
        si = drain_inst.ins.sync_info
        waits = list(si.on_wait) if si and si.on_wait else []
        if len(waits) > 1:
            si.on_wait = waits[:1]
            drain_inst.ins.sync_info = si
            for i in range(1, len(waits)):
                extra = nc.sync.drain()
                esi = extra.ins.sync_info
                upd = list(esi.on_update) if esi and esi.on_update else []
                extra.ins.sync_info = mybir.SyncInfo(
                    on_wait=[waits[i]], on_update=upd
                )
        nc.all_engine_barrier()
        assert self.sems is not None
        popped = nc._tile_sem_poison_stack.pop()
        assert popped is self._sem_poison
        nc.clear_and_free_semaphores(list(self.sems.allocated().values()))
        nc.all_engine_barrier()

    _tile.TileContext._drain_and_barrier = _split_drain_and_barrier

    _orig_add = _tile.TileContext._add_instruction

    def _add_instruction_split(self, inst):
        si = inst.sync_info
        waits = list(si.on_wait) if si and si.on_wait else []
        if len(waits) > 1 and inst.engine != mybir.EngineType.Unassigned:
            for w in waits[:-1]:
                nop = mybir.InstNoOp(
                    name=self.nc.get_next_instruction_name(), ins=[], outs=[]
                )
                nop.engine = inst.engine
                nop.sync_info = mybir.SyncInfo(on_wait=[w], on_update=[])
                _orig_add(self, nop)
            si.on_wait = waits[-1:]
            inst.sync_info = si
        _orig_add(self, inst)

    _tile.TileContext._add_instruction = _add_instruction_split
    _tile.TileContext._drain_split_patched = True


def _build_program(n_tgt, n_src, fsegs, bsegs, taps=False):
    """Build the SPMD bass program (identical on all 8 cores)."""
    from contextlib import ExitStack

    import concourse.bass as bass
    import concourse.mybir as mybir
    import concourse.tile as tile
    from concourse import bacc
    from concourse.masks import make_identity

    _install_drain_patch()

    dt = mybir.dt
    f32 = dt.float32
    i16 = dt.int16
    tb = getattr(dt, _TABLE_DT)
    D = D_FIXED
    DC = D // P
    tgt_sh = n_tgt // NCORES
    src_sh = n_src // NCORES
    NT = tgt_sh // P
    NF = sum(n for seg in fsegs for _, n in seg)
    NB = sum(n for seg in bsegs for _, n in seg)
    AluOp = mybir.AluOpType
    Act = mybir.ActivationFunctionType
    rg = [list(range(NCORES))]

    nc = bacc.Bacc("TRN2", target_bir_lowering=False, debug=False, num_devices=NCORES)

    dram_t = nc.dram_tensor
    HsrcT = dram_t("HsrcT", [D, n_src], f32, kind="ExternalInput").ap()
    W0 = dram_t("W0", [D, D], f32, kind="ExternalInput").ap()
    Wb = dram_t("Wb", [D, D], f32, kind="ExternalInput").ap()
    W1 = dram_t("W1", [D, D], f32, kind="ExternalInput").ap()
    b0_h = dram_t("b0", [1, D], f32, kind="ExternalInput")
    bb_h = dram_t("bb", [1, D], f32, kind="ExternalInput")
    b1_h = dram_t("b1", [1, D], f32, kind="ExternalInput")
    g1T = dram_t("g1T", [P, DC], f32, kind="ExternalInput").ap()
    be1T = dram_t("be1T", [P, DC], f32, kind="ExternalInput").ap()
    g2T = dram_t("g2T", [P, DC], f32, kind="ExternalInput").ap()
    be2T = dram_t("be2T", [P, DC], f32, kind="ExternalInput").ap()
    iota_d = dram_t("iota", [P, P], f32, kind="ExternalInput").ap()
    emb = dram_t("emb", [tgt_sh, D], f32, kind="ExternalInput").ap()
    fe_i16 = dram_t("fe_i16", [P, NF * 8], i16, kind="ExternalInput").ap()
    fe_val = dram_t("fe_val", [P, NF], f32, kind="ExternalInput").ap()
    fe_dl = dram_t("fe_dl", [P, NF], f32, kind="ExternalInput").ap()
    be_i16 = dram_t("be_i16", [P, NB * 8], i16, kind="ExternalInput").ap()
    be_val = dram_t("be_val", [P, NB], f32, kind="ExternalInput").ap()
    be_dl = dram_t("be_dl", [P, NB], f32, kind="ExternalInput").ap()
    outT = dram_t("outT", [D, tgt_sh], f32, kind="ExternalOutput").ap()
    if taps:
        dbg_wh0 = dram_t("dbg_wh0", [n_src, D], tb, kind="ExternalOutput").ap()
        dbg_x1T = dram_t("dbg_x1T", [D, tgt_sh], f32, kind="ExternalOutput").ap()
        dbg_whb = dram_t("dbg_whb", [n_tgt, D], tb, kind="ExternalOutput").ap()
        dbg_wh1 = dram_t("dbg_wh1", [n_src, D], tb, kind="ExternalOutput").ap()
        dbg_st1 = dram_t("dbg_st1", [P, 4], f32, kind="ExternalOutput").ap()
        dbg_x1pre = dram_t("dbg_x1pre", [D, tgt_sh], f32, kind="ExternalOutput").ap()
        dbg_deg = dram_t("dbg_deg", [P, NT], f32, kind="ExternalOutput").ap()

    with tile.TileContext(nc) as tc, ExitStack() as ctx:
        dram = ctx.enter_context(tc.tile_pool(name="dram", bufs=1, space="DRAM"))
        half_rows = n_src // 2
        WH0_t = [dram.tile([half_rows, D], tb, name=f"WH0h{h}") for h in range(2)]
        WHb_loc = dram.tile([tgt_sh, D], tb)
        WHb_full = dram.tile([n_tgt, D], tb, addr_space="Shared")
        WH1_loc = dram.tile([src_sh, D], tb)
        WH1_full = dram.tile([n_src, D], tb, addr_space="Shared")
        WH1_t = [
            WH1_full[h * half_rows : (h + 1) * half_rows, :] for h in range(2)
        ]
        st1_in = dram.tile([P, 2 * DC], f32)
        st1_out = dram.tile([P, 2 * DC], f32, addr_space="Shared")
        st2_in = dram.tile([P, 2 * DC], f32)
        st2_out = dram.tile([P, 2 * DC], f32, addr_space="Shared")

        consts = ctx.enter_context(tc.tile_pool(name="consts", bufs=1))
        w0t = consts.tile([P, DC, D], f32)
        wbt = consts.tile([P, DC, D], f32)
        w1t = consts.tile([P, DC, D], f32)
        for c in range(DC):
            nc.sync.dma_start(out=w0t[:, c, :], in_=W0[c * P : (c + 1) * P, :])
            nc.sync.dma_start(out=wbt[:, c, :], in_=Wb[c * P : (c + 1) * P, :])
            nc.sync.dma_start(out=w1t[:, c, :], in_=W1[c * P : (c + 1) * P, :])
        w0b = consts.tile([P, DC, D], tb)
        wbb = consts.tile([P, DC, D], tb)
        for c in range(DC):
            nc.vector.tensor_copy(out=w0b[:, c, :], in_=w0t[:, c, :])
            nc.vector.tensor_copy(out=wbb[:, c, :], in_=wbt[:, c, :])
        b0bc = consts.tile([P, D], f32)
        bbbc = consts.tile([P, D], f32)
        b1bc = consts.tile([P, D], f32)
        for h_, t_ in ((b0_h, b0bc), (bb_h, bbbc), (b1_h, b1bc)):
            nc.gpsimd.dma_start(
                out=t_[:], in_=bass.AP(tensor=h_, offset=0, ap=[[0, P], [1, D]])
            )
        g1f = consts.tile([P, DC], f32)
        be1f = consts.tile([P, DC], f32)
        g2f = consts.tile([P, DC], f32)
        be2f = consts.tile([P, DC], f32)
        nc.sync.dma_start(out=g1f[:], in_=g1T[:])
        nc.sync.dma_start(out=be1f[:], in_=be1T[:])
        nc.sync.dma_start(out=g2f[:], in_=g2T[:])
        nc.sync.dma_start(out=be2f[:], in_=be2T[:])
        iota_t = consts.tile([P, P], f32)
        nc.sync.dma_start(out=iota_t[:], in_=iota_d[:])
        epst = consts.tile([P, 1], f32)
        nc.vector.memset(epst[:], EPS)
        ident = consts.tile([P, P], f32)
        make_identity(nc, ident[:])
        onesb = consts.tile([P, 1], tb)
        nc.vector.memset(onesb[:], 1.0)

        # resident state
        xT = consts.tile([P, DC, NT, P], tb)  # H_tgt1 (feature-major)
        x2T = consts.tile([P, DC, NT, P], f32)  # layer-1 pre-BN x
        degc = consts.tile([P, NT], f32)  # reciprocal clamped target degree

        sv_pool = ctx.enter_context(tc.tile_pool(name="sv", bufs=8))
        g_pool = ctx.enter_context(tc.tile_pool(name="gp", bufs=3))
        ps_acc = ctx.enter_context(tc.tile_pool(name="psacc", bufs=3, space="PSUM"))
        ps_tr = ctx.enter_context(tc.tile_pool(name="pstr", bufs=2, space="PSUM"))
        hrm = ctx.enter_context(tc.tile_pool(name="hrm", bufs=4))
        whs_pool = ctx.enter_context(tc.tile_pool(name="whs", bufs=4))
        lhs_pool = ctx.enter_context(tc.tile_pool(name="lhs", bufs=2))
        misc = ctx.enter_context(tc.tile_pool(name="misc", bufs=6))

        # ---------------- phase A: full WH0 = H_src @ W0 (every core) ----
        SUP = 512 if n_src % 512 == 0 else P
        for st in range(n_src // SUP):
            haf = lhs_pool.tile([P, SUP], f32, tag="haf")
            hbf = lhs_pool.tile([P, SUP], f32, tag="hbf")
            nc.sync.dma_start(out=haf[:], in_=HsrcT[0:P, st * SUP : (st + 1) * SUP])
            nc.sync.dma_start(
                out=hbf[:], in_=HsrcT[P : 2 * P, st * SUP : (st + 1) * SUP]
            )
            ha = lhs_pool.tile([P, SUP], tb, tag="ha")
            hb = lhs_pool.tile([P, SUP], tb, tag="hb")
            nc.vector.tensor_copy(out=ha[:], in_=haf[:])
            nc.vector.tensor_copy(out=hb[:], in_=hbf[:])
            for r in range(SUP // P):
                ps = ps_acc.tile([P, D], f32, tag="acc")
                nc.tensor.matmul(
                    out=ps[:],
                    lhsT=ha[:, r * P : (r + 1) * P],
                    rhs=w0b[:, 0, :],
                    start=True,
                    stop=False,
                )
                nc.tensor.matmul(
                    out=ps[:],
                    lhsT=hb[:, r * P : (r + 1) * P],
                    rhs=w0b[:, 1, :],
                    start=False,
                    stop=True,
                )
                whs = whs_pool.tile([P, D], tb, tag="whs")
                nc.scalar.copy(out=whs[:], in_=ps[:])
                row0 = (st * (SUP // P) + r) * P
                h = row0 // half_rows
                lr = row0 - h * half_rows
                nc.sync.dma_start(out=WH0_t[h][lr : lr + P, :], in_=whs[:])

        # ---------------- gather pass helper ----------------------------
        def gather_pass(segs, idx_tile, val_tile, dl_tile, tables, want_deg, post):
            """segs: per dst-tile list of (table_idx, ntiles)."""
            j = 0  # global edge-tile index
            for t in range(len(segs)):
                ntile_tot = sum(n for _, n in segs[t])
                ps = ps_acc.tile([P, D], f32, tag="acc", name=f"ps{t}")
                psd = None
                if want_deg:
                    psd = ps_acc.tile([P, 1], f32, tag="deg", bufs=2, name=f"psd{t}")
                k = 0  # tile index within dst-tile
                for tab_i, nseg in segs[t]:
                    table = tables[tab_i]
                    done = 0
                    while done < nseg:
                        bs = min(GBT, nseg - done)
                        gt = g_pool.tile([P, GBT, D], tb, tag="gt", name="gt")
                        nc.gpsimd.dma_gather(
                            out_ap=gt[:, 0:bs, :],
                            in_ap=table,
                            idxs_ap=idx_tile[:, j * 8 : (j + bs) * 8],
                            num_idxs=bs * P,
                            num_idxs_reg=bs * P,
                            elem_size=D,
                            single_packet=False,
                        )
                        for i in range(bs):
                            jj = j + i
                            sv = sv_pool.tile([P, P], tb, name="sv")
                            nc.vector.scalar_tensor_tensor(
                                out=sv[:],
                                in0=iota_t[:],
                                scalar=dl_tile[:, jj : jj + 1],
                                in1=val_tile[:, jj : jj + 1].to_broadcast([P, P]),
                                op0=AluOp.is_equal,
                                op1=AluOp.mult,
                            )
                            nc.tensor.matmul(
                                out=ps[:],
                                lhsT=sv[:],
                                rhs=gt[:, i, :],
                                start=(k + i == 0),
                                stop=(k + i == ntile_tot - 1),
                            )
                            if psd is not None:
                                nc.tensor.matmul(
                                    out=psd[:],
                                    lhsT=sv[:],
                                    rhs=onesb[:],
                                    start=(k + i == 0),
                                    stop=(k + i == ntile_tot - 1),
                                )
                        j += bs
                        done += bs
                        k += bs
                post(t, ps, psd)
            return j

        # ---------------- pass B: layer-0 forward ------------------------
        def post_fwd0(t, ps, psd):
            dtmp = misc.tile([P, 1], f32, tag="dtgt")
            nc.vector.tensor_scalar_max(dtmp[:], psd[:], 1.0)
            nc.vector.reciprocal(degc[:, t : t + 1], dtmp[:])
            h1 = hrm.tile([P, D], f32, tag="h1")
            nc.vector.scalar_tensor_tensor(
                out=h1[:],
                in0=ps[:],
                scalar=degc[:, t : t + 1],
                in1=b0bc[:],
                op0=AluOp.mult,
                op1=AluOp.add,
            )
            h2 = hrm.tile([P, D], f32, tag="h2")
            nc.scalar.activation(out=h2[:], in_=h1[:], func=Act.Relu)
            et = misc.tile([P, D], f32, tag="emb")
            nc.sync.dma_start(out=et[:], in_=emb[t * P : (t + 1) * P, :])
            xr = hrm.tile([P, D], f32, tag="xr")
            nc.vector.tensor_add(xr[:], h2[:], et[:])
            for c in range(DC):
                pt = ps_tr.tile([P, P], f32, tag="tr")
                nc.tensor.transpose(
                    out=pt[:], in_=xr[:, c * P : (c + 1) * P], identity=ident[:]
                )
                nc.vector.tensor_copy(out=xT[:, c, t, :], in_=pt[:])

        with tc.tile_pool(name="edgesB", bufs=1) as ep:
            fidx = ep.tile([P, NF * 8], i16, name="fidxB")
            fval = ep.tile([P, NF], f32, name="fvalB")
            fdl = ep.tile([P, NF], f32, name="fdlB")
            nc.sync.dma_start(out=fidx[:], in_=fe_i16[:])
            nc.sync.dma_start(out=fval[:], in_=fe_val[:])
            nc.sync.dma_start(out=fdl[:], in_=fe_dl[:])
            gather_pass(
                fsegs, fidx, fval, fdl, [t_[:] for t_ in WH0_t], True, post_fwd0
            )

        # ---------------- BN helpers -------------------------------------
        def bn_stats_phase(xbuf, count, st_in_sb_name):
            st_sb = misc.tile([P, 2 * DC], f32, name=st_in_sb_name, tag="stats")
            grp = min(512, count)
            ngrp = count // grp
            for c in range(DC):
                bnst = misc.tile([P, ngrp, 6], f32, tag="bnst")
                flat = xbuf[:, c, :, :].rearrange("p a b -> p (a b)")
                for g in range(ngrp):
                    nc.vector.bn_stats(
                        out=bnst[:, g, :], in_=flat[:, g * grp : (g + 1) * grp]
                    )
                mv = misc.tile([P, 2], f32, tag="mv")
                nc.vector.bn_aggr(out=mv[:], in_=bnst[:].rearrange("p a b -> p (a b)"))
                nc.vector.tensor_scalar_mul(
                    st_sb[:, 2 * c : 2 * c + 1], mv[:, 0:1], float(count)
                )
                musq = misc.tile([P, 1], f32, tag="musq")
                nc.vector.tensor_mul(musq[:], mv[:, 0:1], mv[:, 0:1])
                nc.vector.tensor_add(musq[:], musq[:], mv[:, 1:2])
                nc.vector.tensor_scalar_mul(
                    st_sb[:, 2 * c + 1 : 2 * c + 2], musq[:], float(count)
                )
            return st_sb

        def bn_coeffs(st_full_sb, gamma_f, beta_f, total, a_name, b_name):
            A = misc.tile([P, DC], f32, name=a_name, tag="bnA")
            B = misc.tile([P, DC], f32, name=b_name, tag="bnB")
            for c in range(DC):
                mu = misc.tile([P, 1], f32, tag="mu")
                nc.vector.tensor_scalar_mul(
                    mu[:], st_full_sb[:, 2 * c : 2 * c + 1], 1.0 / total
                )
                q = misc.tile([P, 1], f32, tag="q")
                nc.vector.tensor_scalar_mul(
                    q[:], st_full_sb[:, 2 * c + 1 : 2 * c + 2], 1.0 / total
                )
                musq = misc.tile([P, 1], f32, tag="musq2")
                nc.vector.tensor_mul(musq[:], mu[:], mu[:])
                var = misc.tile([P, 1], f32, tag="var")
                nc.vector.tensor_tensor(
                    out=var[:], in0=q[:], in1=musq[:], op=AluOp.subtract
                )
                sd = misc.tile([P, 1], f32, tag="sd")
                nc.scalar.activation(out=sd[:], in_=var[:], func=Act.Sqrt, bias=epst[:])
                rstd = misc.tile([P, 1], f32, tag="rstd")
                nc.vector.reciprocal(rstd[:], sd[:])
                nc.vector.tensor_mul(A[:, c : c + 1], gamma_f[:, c : c + 1], rstd[:])
                mA = misc.tile([P, 1], f32, tag="mA")
                nc.vector.tensor_mul(mA[:], mu[:], A[:, c : c + 1])
                nc.vector.tensor_tensor(
                    out=B[:, c : c + 1],
                    in0=beta_f[:, c : c + 1],
                    in1=mA[:],
                    op=AluOp.subtract,
                )
            return A, B

        if taps:
            for h in range(2):
                nc.sync.dma_start(
                    out=dbg_wh0[h * half_rows : (h + 1) * half_rows, :],
                    in_=WH0_t[h][:],
                )
            nc.sync.dma_start(out=dbg_deg[:], in_=degc[:])
            for c in range(DC):
                nc.sync.dma_start(
                    out=dbg_x1pre[c * P : (c + 1) * P, :],
                    in_=xT[:, c, :, :].rearrange("p a b -> p (a b)"),
                )

        # ---------------- BN-1 + WHb + AllGather -------------------------
        st1_sb = bn_stats_phase(xT, tgt_sh, "st1_sb")
        nc.sync.dma_start(out=st1_in[:], in_=st1_sb[:])
        nc.gpsimd.collective_compute(
            "AllReduce",
            AluOp.add,
            replica_groups=rg,
            ins=[st1_in[:].opt()],
            outs=[st1_out[:].opt()],
        )
        st1g = misc.tile([P, 2 * DC], f32, tag="stg")
        nc.sync.dma_start(out=st1g[:], in_=st1_out[:])
        A1, B1 = bn_coeffs(st1g, g1f, be1f, n_tgt, "A1", "B1")

        for t in range(NT):
            for c in range(DC):
                nc.vector.scalar_tensor_tensor(
                    out=xT[:, c, t, :],
                    in0=xT[:, c, t, :],
                    scalar=A1[:, c : c + 1],
                    in1=B1[:, c : c + 1].to_broadcast([P, P]),
                    op0=AluOp.mult,
                    op1=AluOp.add,
                )
            ps = ps_acc.tile([P, D], f32, tag="acc", name=f"pswb{t}")
            nc.tensor.matmul(
                out=ps[:], lhsT=xT[:, 0, t, :], rhs=wbb[:, 0, :], start=True, stop=False
            )
            nc.tensor.matmul(
                out=ps[:], lhsT=xT[:, 1, t, :], rhs=wbb[:, 1, :], start=False, stop=True
            )
            whs = whs_pool.tile([P, D], tb, tag="whs")
            nc.scalar.copy(out=whs[:], in_=ps[:])
            nc.sync.dma_start(out=WHb_loc[t * P : (t + 1) * P, :], in_=whs[:])

        nc.gpsimd.collective_compute(
            "AllGather",
            AluOp.bypass,
            replica_groups=rg,
            ins=[WHb_loc[:].opt()],
            outs=[WHb_full[:].opt()],
        )
        if taps:
            nc.sync.dma_start(out=dbg_st1[:], in_=st1_out[:])
            for c in range(DC):
                nc.sync.dma_start(
                    out=dbg_x1T[c * P : (c + 1) * P, :],
                    in_=xT[:, c, :, :].rearrange("p a b -> p (a b)"),
                )
            nc.sync.dma_start(out=dbg_whb[:], in_=WHb_full[:])

        # ---------------- pass E: layer-0 backward (+ fused WH1) ---------
        def post_bwd(t, ps, psd):
            dtmp = misc.tile([P, 1], f32, tag="dsrc")
            nc.vector.tensor_scalar_max(dtmp[:], psd[:], 1.0)
            rtmp = misc.tile([P, 1], f32, tag="rsrc")
            nc.vector.reciprocal(rtmp[:], dtmp[:])
            h1 = hrm.tile([P, D], f32, tag="h1")
            nc.vector.scalar_tensor_tensor(
                out=h1[:],
                in0=ps[:],
                scalar=rtmp[:],
                in1=bbbc[:],
                op0=AluOp.mult,
                op1=AluOp.add,
            )
            h2 = hrm.tile([P, D], f32, tag="h2")
            nc.scalar.activation(out=h2[:], in_=h1[:], func=Act.Relu)
            hsb = misc.tile([P, DC, P], f32, tag="hsb")
            for c in range(DC):
                pt = ps_tr.tile([P, P], f32, tag="tr")
                nc.tensor.transpose(
                    out=pt[:], in_=h2[:, c * P : (c + 1) * P], identity=ident[:]
                )
                nc.vector.tensor_copy(out=hsb[:, c, :], in_=pt[:])
            ps2 = ps_acc.tile([P, D], f32, tag="acc", name=f"psw1{t}")
            nc.tensor.matmul(
                out=ps2[:], lhsT=hsb[:, 0, :], rhs=w1t[:, 0, :], start=True, stop=False
            )
            nc.tensor.matmul(
                out=ps2[:], lhsT=hsb[:, 1, :], rhs=w1t[:, 1, :], start=False, stop=True
            )
            whs = whs_pool.tile([P, D], tb, tag="whs")
            nc.scalar.copy(out=whs[:], in_=ps2[:])
            nc.sync.dma_start(out=WH1_loc[t * P : (t + 1) * P, :], in_=whs[:])

        with tc.tile_pool(name="edgesE", bufs=1) as ep:
            bidx = ep.tile([P, NB * 8], i16, name="bidxE")
            bval = ep.tile([P, NB], f32, name="bvalE")
            bdl = ep.tile([P, NB], f32, name="bdlE")
            nc.sync.dma_start(out=bidx[:], in_=be_i16[:])
            nc.sync.dma_start(out=bval[:], in_=be_val[:])
            nc.sync.dma_start(out=bdl[:], in_=be_dl[:])
            gather_pass(bsegs, bidx, bval, bdl, [WHb_full[:]], True, post_bwd)

        nc.gpsimd.collective_compute(
            "AllGather",
            AluOp.bypass,
            replica_groups=rg,
            ins=[WH1_loc[:].opt()],
            outs=[WH1_full[:].opt()],
        )
        if taps:
            nc.sync.dma_start(out=dbg_wh1[:], in_=WH1_full[:])

        # ---------------- pass G: layer-1 forward ------------------------
        def post_fwd1(t, ps, psd):
            h1 = hrm.tile([P, D], f32, tag="h1")
            nc.vector.scalar_tensor_tensor(
                out=h1[:],
                in0=ps[:],
                scalar=degc[:, t : t + 1],
                in1=b1bc[:],
                op0=AluOp.mult,
                op1=AluOp.add,
            )
            h2 = hrm.tile([P, D], f32, tag="h2")
            nc.scalar.activation(out=h2[:], in_=h1[:], func=Act.Relu)
            for c in range(DC):
                pt = ps_tr.tile([P, P], f32, tag="tr")
                nc.tensor.transpose(
                    out=pt[:], in_=h2[:, c * P : (c + 1) * P], identity=ident[:]
                )
                nc.vector.tensor_add(x2T[:, c, t, :], pt[:], xT[:, c, t, :])

        with tc.tile_pool(name="edgesG", bufs=1) as ep:
            fidx2 = ep.tile([P, NF * 8], i16, name="fidxG")
            fval2 = ep.tile([P, NF], f32, name="fvalG")
            fdl2 = ep.tile([P, NF], f32, name="fdlG")
            nc.sync.dma_start(out=fidx2[:], in_=fe_i16[:])
            nc.sync.dma_start(out=fval2[:], in_=fe_val[:])
            nc.sync.dma_start(out=fdl2[:], in_=fe_dl[:])
            gather_pass(
                fsegs, fidx2, fval2, fdl2, [t_[:] for t_ in WH1_t], False, post_fwd1
            )

        # ---------------- BN-2 + output ----------------------------------
        st2_sb = bn_stats_phase(x2T, tgt_sh, "st2_sb")
        nc.sync.dma_start(out=st2_in[:], in_=st2_sb[:])
        nc.gpsimd.collective_compute(
            "AllReduce",
            AluOp.add,
            replica_groups=rg,
            ins=[st2_in[:].opt()],
            outs=[st2_out[:].opt()],
        )
        st2g = misc.tile([P, 2 * DC], f32, tag="stg")
        nc.sync.dma_start(out=st2g[:], in_=st2_out[:])
        A2, B2 = bn_coeffs(st2g, g2f, be2f, n_tgt, "A2", "B2")

        for t in range(NT):
            for c in range(DC):
                nc.vector.scalar_tensor_tensor(
                    out=x2T[:, c, t, :],
                    in0=x2T[:, c, t, :],
                    scalar=A2[:, c : c + 1],
                    in1=B2[:, c : c + 1].to_broadcast([P, P]),
                    op0=AluOp.mult,
                    op1=AluOp.add,
                )
        for c in range(DC):
            nc.sync.dma_start(
                out=outT[c * P : (c + 1) * P, :],
                in_=x2T[:, c, :, :].rearrange("p a b -> p (a b)"),
            )

    nc.compile()
    return nc


# ----------------------------------------------------------------- entry


def _run(inputs, trace=False, tmpdir=None, taps=False):
    from concourse.bass_utils import run_bass_kernel_spmd

    H_src = np.asarray(inputs["H_src"], dtype=np.float32)
    target_emb = np.asarray(inputs["target_emb"], dtype=np.float32)
    W_fwd = np.asarray(inputs["W_fwd"], dtype=np.float32)
    b_fwd = np.asarray(inputs["b_fwd"], dtype=np.float32)
    W_bwd = np.asarray(inputs["W_bwd"], dtype=np.float32)
    b_bwd = np.asarray(inputs["b_bwd"], dtype=np.float32)
    gamma = np.asarray(inputs["gamma"], dtype=np.float32)
    beta = np.asarray(inputs["beta"], dtype=np.float32)
    vals = np.asarray(inputs["vals"], dtype=np.float32)
    rows = np.asarray(inputs["rows"])
    cols = np.asarray(inputs["cols"])

    n_src, D = H_src.shape
    n_tgt = target_emb.shape[0]
    assert D == D_FIXED
    tgt_sh = n_tgt // NCORES
    DC = D // P

    fsegs, f_i, f_v, f_d = _edge_plan(
        rows, cols, vals, n_tgt, n_src, NCORES, split=True
    )
    bsegs, b_i, b_v, b_d = _edge_plan(
        cols, rows, vals, n_src, n_tgt, NCORES, split=False
    )

    nc = _build_program(n_tgt, n_src, fsegs, bsegs, taps=taps)

    HsrcT = np.ascontiguousarray(H_src.T)
    iota = np.ascontiguousarray(np.tile(np.arange(P, dtype=np.float32), (P, 1)))

    def fmaj(v):  # [D] -> [P, DC] feature-major
        return np.ascontiguousarray(v.reshape(DC, P).T)

    in_maps = []
    for c in range(NCORES):
        in_maps.append(
            {
                "HsrcT": HsrcT,
                "W0": W_fwd[0],
                "Wb": W_bwd[0],
                "W1": W_fwd[1],
                "b0": b_fwd[0].reshape(1, D),
                "bb": b_bwd[0].reshape(1, D),
                "b1": b_fwd[1].reshape(1, D),
                "g1T": fmaj(gamma[0]),
                "be1T": fmaj(beta[0]),
                "g2T": fmaj(gamma[1]),
                "be2T": fmaj(beta[1]),
                "iota": iota,
                "emb": np.ascontiguousarray(
                    target_emb[c * tgt_sh : (c + 1) * tgt_sh]
                ),
                "fe_i16": f_i[c],
                "fe_val": f_v[c],
                "fe_dl": f_d[c],
                "be_i16": b_i[c],
                "be_val": b_v[c],
                "be_dl": b_d[c],
            }
        )

    res = run_bass_kernel_spmd(
        nc, in_maps, list(range(NCORES)), trace=trace, tmpdir=tmpdir
    )
    out = np.concatenate(
        [np.asarray(res.results[c]["outT"]).astype(np.float32).T for c in range(NCORES)],
        axis=0,
    )
    return np.ascontiguousarray(out), res


def kernel(**inputs) -> np.ndarray:
    out, _ = _run(inputs)
    return out
